# revision 54
# baseline (speedup 1.0000x reference)
"""5G Polar encoder (CRC11 + subchannel alloc + butterfly + interleave) on 8 trn2 cores.

The whole reference computation is GF(2)-linear in u:
    parity  = (u @ crc_gen) mod 2                       -> linear
    bits    = [u | parity] = u @ [I | crc_gen]          -> linear
    scatter x[:, info_pos] = bits                       -> column selection (linear)
    butterfly stages x ^= x[:, g[s]]                    -> linear over GF(2)
    out     = x[:, perm_out]                            -> column gather (linear)

So on the host we compose one binary matrix M [512, 1024] from the tiny index
tables (cheap uint8 ops), and the device kernel is a single fused
    y = (u @ M) mod 2
data-parallel over the batch: each of the 8 cores computes an [8192, 512] @
[512, 1024] matmul in fp8e4 with DoubleRow perf mode (exact: all values are
0/1, sums <= 523 accumulate in f32 PSUM).

Active design (VERSION=51 = _build_nc_v7 + V7_OPTS_B, ~72.0-72.6us/core
vs 74.1us for the previous v3/VERSION=20 design; NTFF min-of-3; rel err 0):
  - exec_time window = [first const MEMSET (~5.9us, framework preamble)
    .. end of the walrus-emitted teardown]. The teardown (zeroes all 255
    HW semaphores across 5 engines + barrier rounds) is ~6.7-8.7us and
    NOT controllable from kernel code; a trivial 1-copy kernel measures
    ~13us. Budget: ~5.4 front + 55.2 MM phase + ~2.2 dither + 2.6 tail
    + ~1.1 waits + ~7.6 teardown.
  - Input = ONE host-packed blob [128, 36864B] per core:
    [c0|mt01|mt23|c1|c2|c3|c4|c5|c6]. [c0|mt01|mt23] fetched as a single
    583KB DMA (one handoff, one sem -> tile0 fully ready ~10.8us), then
    c1, c2, c3..c6 as separate DMAs, all on the single SP queue. Within
    a queue transfers are strictly FIFO (no bandwidth stealing) but
    EVERY DMA instruction costs ~0.5-1.7us of dead handoff before its
    packets flow -> merge small early loads; 26 DMA instrs (v3 had 74).
  - Outputs grouped 4 b-tiles per DMA with DRAM layout [16,128,4096]
    (partition rows of 4 tiles contiguous -> 4KB packets ~300GB/s; 1KB
    row packets only sustain ~180GB/s, which backlogged v3's out queue).
    Host un-groups with a cheap transpose in kernel(). Last group's DMA
    split (2048,1024,1024) cols, each span emitted as its tiles evict;
    last two tiles' evictions split ACT/DVE halves.
  - PSUM halves [128,512] f32, bufs=8 (1 bank each): finer release
    granularity; ks-outer MM order (mt23 first needed at MM3); h0 half
    always evicts on ACT, h1 on DVE (~620ns each per 864ns tile). Raw
    sums out as u8 (saturation certificate: no 255 => exact, else rerun
    i16 build); host does &1.
  - 28 contiguous warmup MMs (fd=128, ~120ns each) from ~7.2us: the PE
    clock ramps to 2.4GHz only after ~3.5us of CONTIGUOUS PE activity
    (idle gaps reset it; cold MMs run 427ns vs 216 warm). Small scratch
    [128,2,128] memset (~300ns) so warmups start early.

Hard-won HW facts (measured on this machine; keep for future sessions):
  - fp8 DoubleRow 216ns/MM (N=512) is the PE floor: 157 TF/s cap. All
    Double* perf modes cap at 2x; no fp4/quad mode exists. Butterfly/
    Kronecker decompositions don't beat the dense GEMM: PE cycles =
    K-granules(256) x N-columns and rank(M)=512 forces 2 granules.
  - A fixed ~432ns PE stall recurs every 10.791us in EVERY build
    (clock-management dither, unavoidable, ~2.2us per run).
  - add_dep_helper(dma_inst, mm_inst) deps flipped the whole core into
    a 2.0GHz state (ALL engines 1.2x slower, 3/3 runs) - do not use.
  - Multi-queue DMA (scalar/gpsimd HWDGE): each extra queue adds ~1.1us
    teardown, and queues compete per-packet round-robin (big packets
    win, no prioritization) - single SP queue + FIFO order is better.
  - The Tile scheduler reorders same-queue DMAs that have no deps;
    emission order does NOT pin issue order.
  - tensor_scalar `mod` and ACT `Sin` don't work on HW; Pool (gpsimd)
    copies are ~4.25us/tile; fused AND+cast rejected ("TSP bitVec op
    cannot do cast"); warmup-on-uninitialized-SBUF rejected by Tile
    ("Releasing unallocated Tile ... read but not written").
"""

import numpy as np
import ml_dtypes

N_CORES = 8
BS = 65536
K = 512          # u feature dim (contraction)
N = 1024         # output columns
SHARD = BS // N_CORES  # 8192 batch rows per core
P = 128
KT = K // P      # 4 k-tiles
NB = SHARD // P  # 64 batch tiles per core

FP8_NP = ml_dtypes.float8_e4m3

_nc_cache = {}


def build_M(crc_gen, info_pos, ind_gather, perm_out):
    """Compose the encoder into one GF(2) matrix M [K, N]: out = (u @ M) mod 2."""
    crc_gen = np.asarray(crc_gen)
    info_pos = np.asarray(info_pos)
    ind_gather = np.asarray(ind_gather)
    perm_out = np.asarray(perm_out)
    k, _ = crc_gen.shape
    nb, n1 = ind_gather.shape
    kp = info_pos.shape[0]
    C = (crc_gen.astype(np.int64) & 1).astype(np.uint8)
    B = np.concatenate([np.eye(k, dtype=np.uint8), C], axis=1)  # [k, kp]
    # scatter bits into columns; duplicate indices: last write wins (matches
    # jax/numpy .at[].set application order)
    col_src = np.full(n1, -1, np.int64)
    col_src[info_pos] = np.arange(kp)
    A = np.zeros((k, n1), np.uint8)
    valid = col_src >= 0
    A[:, valid] = B[:, col_src[valid]]
    for s in range(nb):
        A = A ^ A[:, ind_gather[s]]
    return A[:, perm_out]  # [k, n]


def _build_nc(reps=1, do_mm=True, do_evict=True, evict="pool",
              w1_act=64, w3_dve=0, ev_stage=3, u_chunks=1, wbufs=4,
              ks_outer=False):
    """evict modes:
    - "pool":    ACT f32->i16, DVE AND, Pool narrow i16->i8, DMA i8
    - "dve":     ACT f32->i16, DVE AND, DVE narrow i16->i8, DMA i8
    - "i16out":  ACT f32->i16, DVE AND, DMA out i16 (host takes low bits)
    - "dmacast": ACT f32->i16, DVE AND, gpsimd casting DMA i16->i8
    - "split":   W1 on ACT for w1_act tiles/64 else DVE; AND on DVE;
                 narrow on DVE for w3_dve tiles/64 else Pool; DMA i8
    """
    import concourse.tile as tile
    from concourse import bacc, mybir

    nc = bacc.Bacc("TRN2", target_bir_lowering=False, debug=False)
    fp8 = mybir.dt.float8e4
    f32 = mybir.dt.float32
    i16 = mybir.dt.int16
    i8 = mybir.dt.int8
    DR = mybir.MatmulPerfMode.DoubleRow

    # k-major 3D layouts: [p, ks, free] with global k = ks*128 + p (both
    # operands use the same mapping, so the contraction is correct).
    uT = nc.declare_dram_parameter("uT", [P, KT, SHARD], fp8, isOutput=False)
    mat = nc.declare_dram_parameter("mat", [P, KT, N], fp8, isOutput=False)
    y_dt = i16 if evict == "i16out" else i8
    y = nc.declare_dram_parameter("y", [SHARD, N], y_dt, isOutput=True)

    with tile.TileContext(nc) as tc:
        with (
            tc.tile_pool(name="consts", bufs=1) as cpool,
            tc.tile_pool(name="work", bufs=wbufs) as wpool,
            tc.tile_pool(name="outs", bufs=4) as opool,
            tc.tile_pool(name="psum", bufs=4, space="PSUM") as ppool,
        ):
            mt = cpool.tile([P, KT, N], fp8, tag="mt")
            nc.sync.dma_start(mt[:], mat[:])
            # chunk the big u load along batch so the first b-tile's matmuls
            # start after ~1/u_chunks of the 4MB has landed
            CW = SHARD // u_chunks
            uts = []
            for c in range(u_chunks):
                ut_c = cpool.tile([P, KT, CW], fp8, tag=f"ut{c}", name=f"ut{c}")
                nc.sync.dma_start(ut_c[:], uT[:, :, c * CW:(c + 1) * CW])
                uts.append(ut_c)
            ot_shared = None
            if evict == "outonly":
                ot_shared = cpool.tile([P, N], i8, tag="ot_shared")
                nc.any.memset(ot_shared[:], 0)
            ps_shared = None
            if not do_mm:
                ps_shared = ppool.tile([P, N], f32, tag="ps_shared")
                for h in range(2):
                    nc.tensor.matmul(
                        ps_shared[:, h * 512:(h + 1) * 512],
                        uts[0][:, 0:2, 0:P],
                        mt[:, 0:2, h * 512:(h + 1) * 512],
                        start=True, stop=True, perf_mode=DR,
                    )
            for i, b in enumerate(
                [b for _ in range(reps) for b in range(NB)]
            ):
                if do_mm:
                    ps = ppool.tile([P, N], f32, tag="ps", name="ps")
                else:
                    ps = ps_shared
                t16 = wpool.tile([P, N], i16, tag="t16")
                a16 = wpool.tile([P, N], i16, tag="a16")
                ot = opool.tile([P, N], i8, tag="ot")
                if do_mm:
                    ut = uts[(b * P) // CW]
                    boff = (b * P) % CW
                    loop = (
                        [(h, ks) for ks in range(0, KT, 2) for h in range(2)]
                        if ks_outer else
                        [(h, ks) for h in range(2) for ks in range(0, KT, 2)]
                    )
                    for h, ks in loop:
                        nc.tensor.matmul(
                            ps[:, h * 512:(h + 1) * 512],
                            ut[:, ks:ks + 2, boff:boff + P],
                            mt[:, ks:ks + 2, h * 512:(h + 1) * 512],
                            start=(ks == 0),
                            stop=(ks == KT - 2),
                            perf_mode=DR,
                            skip_group_check=ks_outer,
                        )
                if do_evict:
                    if evict == "outonly":
                        nc.sync.dma_start(y[b * P:(b + 1) * P, :], ot_shared[:])
                        continue
                    # W1: PSUM f32 -> i16
                    if ev_stage >= 1:
                        if evict == "w1dve" or (i % NB) >= w1_act:
                            nc.vector.tensor_copy(t16[:], ps[:])
                        else:
                            nc.scalar.activation(
                                t16[:], ps[:],
                                mybir.ActivationFunctionType.Copy,
                            )
                    # W2: AND with 1
                    if ev_stage >= 2:
                        nc.vector.tensor_scalar(
                            a16[:], t16[:], 1, None,
                            mybir.AluOpType.bitwise_and,
                        )
                    # W3 + output DMA
                    if ev_stage < 3:
                        continue
                    if evict == "i16out":
                        nc.sync.dma_start(y[b * P:(b + 1) * P, :], a16[:])
                    elif evict in ("dmacast", "w1dve"):
                        nc.gpsimd.dma_start(y[b * P:(b + 1) * P, :], a16[:])
                    else:
                        if evict == "dve" or (
                            evict == "split" and (i % NB) < w3_dve
                        ):
                            nc.vector.tensor_copy(ot[:], a16[:])
                        else:
                            nc.gpsimd.tensor_copy(ot[:], a16[:])
                        nc.sync.dma_start(y[b * P:(b + 1) * P, :], ot[:])
    nc.compile()
    return nc


def _build_nc_v2(reps=1, act_pairs=22, warm=(256, 256), main_chunk=1024,
                 chunk_bufs=3, wbufs=3, mt_splits=4):
    """v2: pair eviction ([128,2048] f32 = 4 PSUM banks per evict instr),
    i8 output, W1 split ACT/DVE, staged input DMA with pool backpressure.

    Per pair (2 b-tiles): 8 matmuls fill 4 banks; one W1 (PSUM f32->i16,
    ACT for act_pairs/32 of pairs else DVE), one DVE AND (i16), one DVE
    narrow (i16->i8, safe post-AND), 2 output DMAs.
    """
    import concourse.tile as tile
    from concourse import bacc, mybir

    nc = bacc.Bacc("TRN2", target_bir_lowering=False, debug=False)
    fp8 = mybir.dt.float8e4
    f32 = mybir.dt.float32
    i16 = mybir.dt.int16
    i8 = mybir.dt.int8
    DR = mybir.MatmulPerfMode.DoubleRow

    uT = nc.declare_dram_parameter("uT", [P, KT, SHARD], fp8, isOutput=False)
    mat = nc.declare_dram_parameter("mat", [P, KT, N], fp8, isOutput=False)
    # raw i16 sums; host computes & 1
    y = nc.declare_dram_parameter("y", [SHARD, N], i16, isOutput=True)

    # batch chunk schedule: warmup chunks then fixed-size main chunks
    chunks = list(warm)
    while sum(chunks) < SHARD:
        chunks.append(min(main_chunk, SHARD - sum(chunks)))
    starts = [sum(chunks[:i]) for i in range(len(chunks))]

    PAIRS = NB // 2

    with tile.TileContext(nc) as tc:
        with (
            tc.tile_pool(name="consts", bufs=1) as cpool,
            tc.tile_pool(name="uchunks", bufs=chunk_bufs) as upool,
            tc.tile_pool(name="work", bufs=wbufs) as wpool,
            tc.tile_pool(name="outs", bufs=wbufs) as opool,
            tc.tile_pool(name="psum", bufs=2, space="PSUM") as ppool,
        ):
            # mt as one DMA: [P, KT*N] rows are 4KB contiguous -> big packets
            mt = cpool.tile([P, KT, N], fp8, tag="mt")
            nc.sync.dma_start(mt[:], mat[:])
            # u chunk tiles from a small pool: chunk c+chunk_bufs's DMA
            # waits for chunk c's matmuls (natural backpressure keeps
            # early chunks from sharing DMA bandwidth with late ones)
            chunk_map = {}  # b-tile index -> (tile, local col offset)
            pending = list(zip(starts, chunks))

            def prefetch(upto_tile):
                # emit chunk DMAs for chunks whose first b-tile <= upto_tile;
                # warmup chunks come from consts pool (no reuse), main chunks
                # from upool (bufs=chunk_bufs gives DMA backpressure)
                for st, cw in pending[:]:
                    if st // P > upto_tile:
                        break
                    wi = starts.index(st)
                    pool = cpool if wi < len(warm) else upool
                    t = pool.tile([P, KT, cw], fp8,
                                  tag=("uw%d" % wi if wi < len(warm) else "uc"),
                                  name=f"uc{st}")
                    nc.sync.dma_start(t[:], uT[:, :, st:st + cw])
                    for bb in range(st // P, (st + cw) // P):
                        chunk_map[bb] = (t, bb * P - st)
                    pending.remove((st, cw))

            PF = 8  # prefetch distance in b-tiles

            for it in range(reps):
                for i in range(PAIRS):
                    prefetch(2 * i + 1 + PF)
                    ps = ppool.tile([P, 2 * N], f32, tag="ps", name="ps")
                    for t in range(2):
                        b = 2 * i + t
                        ut, boff = chunk_map[b]
                        for ks in range(0, KT, 2):
                            for h in range(2):
                                nc.tensor.matmul(
                                    ps[:, t * N + h * 512:
                                       t * N + (h + 1) * 512],
                                    ut[:, ks:ks + 2, boff:boff + P],
                                    mt[:, ks:ks + 2, h * 512:(h + 1) * 512],
                                    start=(ks == 0),
                                    stop=(ks == KT - 2),
                                    perf_mode=DR,
                                    skip_group_check=True,
                                )
                    t16 = wpool.tile([P, 2, N], i16, tag="t16")
                    # Bresenham split of W1 between ACT and DVE; raw sums
                    # go straight out (host does & 1)
                    on_act = (i * act_pairs) % PAIRS < act_pairs
                    if on_act:
                        nc.scalar.activation(
                            t16[:], ps[:],
                            mybir.ActivationFunctionType.Copy)
                    else:
                        nc.vector.tensor_copy(t16[:], ps[:])
                    for t in range(2):
                        b = 2 * i + t
                        nc.sync.dma_start(y[b * P:(b + 1) * P, :], t16[:, t])
    nc.compile()
    return nc


def chunk_schedule(warm, main_chunk):
    chunks = list(warm)
    while sum(chunks) < SHARD:
        chunks.append(min(main_chunk, SHARD - sum(chunks)))
    starts = [sum(chunks[:i]) for i in range(len(chunks))]
    return starts, chunks


def _build_nc_v3(reps=1, warm=(256, 256, 512), main_chunk=1024,
                 chunk_bufs=3, wbufs=3, warmup_mms=40, pf=8, out_u8=True,
                 pair_dma=False, chunk_major=False, warmup_fd=512,
                 split_mt=False, alt_out_queue=False, fast_tail=0,
                 warmup_noinit=False):
    """v3: pair PSUM ([128,2048] f32, bufs=2) with W1 split across BOTH
    engines per pair (ACT evicts tile A's 1024 cols, DVE tile B's) so the
    pair frees in ~1.4us < the 2.1us matmul fill time -> PE never stalls.
    Raw i16 sums out (host does &1). Dummy warmup matmuls during the input
    lead-in keep the PE's HAM clock at 2.4GHz for the first real tiles.
    """
    import concourse.tile as tile
    from concourse import bacc, mybir

    nc = bacc.Bacc("TRN2", target_bir_lowering=False, debug=False)
    fp8 = mybir.dt.float8e4
    f32 = mybir.dt.float32
    i16 = mybir.dt.int16
    DR = mybir.MatmulPerfMode.DoubleRow

    u8 = mybir.dt.uint8
    out_dt = u8 if out_u8 else i16

    # chunk_major: host lays u out chunk-contiguous ([P, KT*cw] per chunk,
    # concatenated) so each chunk DMA is one contiguous run per partition
    uT = nc.declare_dram_parameter(
        "uT", [P, KT * SHARD] if chunk_major else [P, KT, SHARD], fp8,
        isOutput=False)
    mat = nc.declare_dram_parameter("mat", [P, KT, N], fp8, isOutput=False)
    # raw sums out: u8 saturating (host certifies no 255 appeared -> exact,
    # else reruns the i16 build) or i16 exact
    y = nc.declare_dram_parameter("y", [SHARD, N], out_dt, isOutput=True)

    starts, chunks = chunk_schedule(warm, main_chunk)
    PAIRS = NB // 2

    with tile.TileContext(nc) as tc:
        with (
            tc.tile_pool(name="consts", bufs=1) as cpool,
            tc.tile_pool(name="uchunks", bufs=chunk_bufs) as upool,
            tc.tile_pool(name="work", bufs=wbufs) as wpool,
            tc.tile_pool(name="psum", bufs=4, space="PSUM") as ppool,
        ):
            # PE warmup: dummy matmuls with no DMA deps keep the HAM busy
            # window hot while inputs stream in. Scratch operands from a
            # memset tile (DVE memsets it right after the preamble); results
            # land in a psum buf that a later tile overwrites (start=True).
            # warmup_fd tunes per-MM duration so the warmup block ends just
            # as the first input chunk lands (queue order gates real MMs).
            if warmup_mms:
                wfd = max(warmup_fd, P)
                scratch = cpool.tile([P, 2, wfd], fp8, tag="scratch")
                if not warmup_noinit:
                    nc.vector.memset(scratch[:], 0)
                # warmup_noinit: read uninitialized SBUF (garbage values are
                # fine -- warmup psum results are discarded and overwritten
                # with start=True) so the PE starts ~2us earlier, right
                # after its own preamble instead of after DVE's memset
                wp = ppool.tile([P, N], f32, tag="ps", name="ps_warm")
                for _ in range(warmup_mms):
                    nc.tensor.matmul(wp[:, 0:wfd], scratch[:, :, 0:P],
                                     scratch[:], start=True, stop=True,
                                     perf_mode=DR, skip_group_check=True)

            # mt split by ks-pairs: the first tiles' start-group matmuls only
            # need ks 0-1 (256KB), so they launch ~1.3us before the full
            # 512KB would have landed; ks 2-3 arrives while they run
            chunk_map = {}
            pending = list(zip(starts, chunks))

            if split_mt:
                mt01 = cpool.tile([P, 2, N], fp8, tag="mt01")
                mt23 = cpool.tile([P, 2, N], fp8, tag="mt23")
                if split_mt == "fine":
                    # two parallel DMAs for mt01 double its share of the
                    # round-robin DMA ring bandwidth -> first matmul earlier
                    nc.sync.dma_start(mt01[:, 0:1, :], mat[:, 0:1, :])
                    nc.sync.dma_start(mt01[:, 1:2, :], mat[:, 1:2, :])
                elif split_mt == "h":
                    # h-halves: tile 0's first matmul reads only cols 0-511
                    # of mt01 (region-tracked), gating on 128KB not 256KB
                    nc.sync.dma_start(mt01[:, :, 0:512], mat[:, 0:2, 0:512])
                else:
                    nc.sync.dma_start(mt01[:], mat[:, 0:2, :])
                mt_of = {0: (mt01, 0), 2: (mt23, 0)}
            else:
                mt = cpool.tile([P, KT, N], fp8, tag="mt")
                nc.sync.dma_start(mt[:], mat[:])
                mt_of = {0: (mt, 0), 2: (mt, 2)}

            def prefetch(upto_tile):
                for st, cw in pending[:]:
                    if st // P > upto_tile:
                        break
                    wi = starts.index(st)
                    pool = cpool if wi < len(warm) else upool
                    t = pool.tile([P, KT, cw], fp8,
                                  tag=("uw%d" % wi if wi < len(warm) else "uc"),
                                  name=f"uc{st}")
                    if chunk_major:
                        off = KT * st
                        src = uT[:, off:off + KT * cw].rearrange(
                            "p (k c) -> p k c", k=KT)
                    else:
                        src = uT[:, :, st:st + cw]
                    nc.sync.dma_start(t[:], src)
                    for bb in range(st // P, (st + cw) // P):
                        chunk_map[bb] = (t, bb * P - st)
                    pending.remove((st, cw))

            if split_mt:
                prefetch(0)  # chunk0 lands right behind mt01
                if split_mt == "h":
                    nc.sync.dma_start(mt01[:, :, 512:N], mat[:, 0:2, 512:N])
                nc.sync.dma_start(mt23[:], mat[:, 2:4, :])

            for it in range(reps):
                for b in range(NB):
                    prefetch(b + pf)
                    ps = ppool.tile([P, N], f32, tag="ps", name="ps")
                    ut, boff = chunk_map[b]
                    for ks in range(0, KT, 2):
                        mtt, mks = mt_of[ks]
                        for h in range(2):
                            nc.tensor.matmul(
                                ps[:, h * 512:(h + 1) * 512],
                                ut[:, ks:ks + 2, boff:boff + P],
                                mtt[:, mks:mks + 2, h * 512:(h + 1) * 512],
                                start=(ks == 0),
                                stop=(ks == KT - 2),
                                perf_mode=DR,
                                skip_group_check=True,
                            )
                    # W1 alternates engines per tile; 4-deep psum pipeline
                    # absorbs eviction latency jitter
                    if pair_dma:
                        if b % 2 == 0:
                            t16p = wpool.tile([P, 2, N], out_dt, tag="t16")
                            nc.scalar.activation(
                                t16p[:, 0], ps[:],
                                mybir.ActivationFunctionType.Copy)
                        else:
                            nc.vector.tensor_copy(t16p[:, 1], ps[:])
                            dst = y[(b - 1) * P:(b + 1) * P, :].rearrange(
                                "(t p) n -> p t n", t=2)
                            nc.sync.dma_start(dst, t16p[:])
                        continue
                    t16 = wpool.tile([P, N], out_dt, tag="t16")
                    if b >= NB - fast_tail:
                        # tail tiles: split the evict across BOTH engines
                        # (different psum banks) + 2 half-DMAs so the final
                        # serial chain after the last matmul is shorter
                        nc.scalar.activation(t16[:, 0:512], ps[:, 0:512],
                                             mybir.ActivationFunctionType.Copy)
                        nc.vector.tensor_copy(t16[:, 512:N], ps[:, 512:N])
                        nc.sync.dma_start(y[b * P:(b + 1) * P, 0:512],
                                          t16[:, 0:512])
                        nc.sync.dma_start(y[b * P:(b + 1) * P, 512:N],
                                          t16[:, 512:N])
                        continue
                    if b % 2 == 0:
                        nc.scalar.activation(t16[:], ps[:],
                                             mybir.ActivationFunctionType.Copy)
                    else:
                        nc.vector.tensor_copy(t16[:], ps[:])
                    # odd tiles' out-DMA issues from the ACT queue (HWDGE on
                    # either SP or ACT) -> halves SP descriptor pacing
                    eng = nc.scalar if (alt_out_queue and b % 2 == 1) else nc.sync
                    eng.dma_start(y[b * P:(b + 1) * P, :], t16[:])
    nc.compile()
    return nc


def _build_nc_v4(reps=1, warm=(128, 256, 512), main_chunk=2048,
                 chunk_bufs=3, wbufs=8, warmup_mms=5, pf=12, out_u8=True,
                 warmup_fd=512, warmup_noinit=False, fast_tail=2,
                 chunk_major=True, mt_q="scalar", chunk_q="gpsimd",
                 out_q=("sync", "gpsimd"), split_mt="h", psum_bufs=4,
                 dummy_dma=False, evict_split="alt"):
    """v4/v5 experiments on top of v3.

    Measured v4 lesson (multi-queue: mt on ACT, chunks on Pool, outs on
    SP+Pool): DMA engines round-robin across ALL queues with pending
    descriptors, so extra queues give no prioritization (v3's single-queue
    FIFO order IS the priority mechanism), and every extra HWDGE queue
    adds ~1.1us to the fixed NEFF teardown (postamble queue reset). ->
    v5 reverts to a single SP queue for everything.

    v5 additions:
      - dummy_dma: a 1-byte DMA as the first SP op wakes the DMA engine
        rings (~0.8us spin-up) during the descriptor gen of the real
        first loads.
      - chunk_major: u laid out chunk-contiguous so warm-chunk DMA
        packets are >=512B (the [P,KT,cw] layout gives cw-byte packets:
        128B for the first warm chunk, ~5GB/s/engine).
      - evict_split="tile": EVERY tile's eviction splits into ACT half +
        DVE half (~630ns each) instead of alternating whole-tile
        evictions (1114/1224ns): mid-phase traces show PSUM-release
        backpressure stalls (MM waits on eviction sems) with the
        alternating scheme.
    """
    import concourse.tile as tile
    from concourse import bacc, mybir

    nc = bacc.Bacc("TRN2", target_bir_lowering=False, debug=False)
    fp8 = mybir.dt.float8e4
    f32 = mybir.dt.float32
    i16 = mybir.dt.int16
    DR = mybir.MatmulPerfMode.DoubleRow

    u8 = mybir.dt.uint8
    out_dt = u8 if out_u8 else i16

    uT = nc.declare_dram_parameter(
        "uT", [P, KT * SHARD] if chunk_major else [P, KT, SHARD], fp8,
        isOutput=False)
    mat = nc.declare_dram_parameter("mat", [P, KT, N], fp8, isOutput=False)
    y = nc.declare_dram_parameter("y", [SHARD, N], out_dt, isOutput=True)

    starts, chunks = chunk_schedule(warm, main_chunk)

    with tile.TileContext(nc) as tc:
        eng = {"sync": nc.sync, "scalar": nc.scalar, "vector": nc.vector,
               "gpsimd": nc.gpsimd, "tensor": nc.tensor}
        mtq = eng[mt_q]
        ckq = eng[chunk_q]
        oq0, oq1 = eng[out_q[0]], eng[out_q[1]]
        with (
            tc.tile_pool(name="consts", bufs=1) as cpool,
            tc.tile_pool(name="uchunks", bufs=chunk_bufs) as upool,
            tc.tile_pool(name="work", bufs=wbufs) as wpool,
            tc.tile_pool(name="psum", bufs=psum_bufs, space="PSUM") as ppool,
        ):
            if dummy_dma:
                # 1-byte DMA as the first queue op: rings spin up (~0.8us)
                # while the real loads' descriptors generate
                wake = cpool.tile([1, 1], fp8, tag="wake")
                mtq.dma_start(wake[:], mat[0:1, 0:1, 0:1])
            # mt h-split so tile0's first matmul gates on 128KB
            # (region-level tracking), rest streams behind
            mt01 = cpool.tile([P, 2, N], fp8, tag="mt01")
            mt23 = cpool.tile([P, 2, N], fp8, tag="mt23")
            if split_mt == "h":
                mtq.dma_start(mt01[:, :, 0:512], mat[:, 0:2, 0:512])
            else:
                mtq.dma_start(mt01[:], mat[:, 0:2, :])
            mt_of = {0: (mt01, 0), 2: (mt23, 0)}

            chunk_map = {}
            pending = list(zip(starts, chunks))

            def prefetch(upto_tile):
                for st, cw in pending[:]:
                    if st // P > upto_tile:
                        break
                    wi = starts.index(st)
                    pool = cpool if wi < len(warm) else upool
                    t = pool.tile([P, KT, cw], fp8,
                                  tag=("uw%d" % wi if wi < len(warm) else "uc"),
                                  name=f"uc{st}")
                    if chunk_major:
                        off = KT * st
                        src = uT[:, off:off + KT * cw].rearrange(
                            "p (k c) -> p k c", k=KT)
                    else:
                        src = uT[:, :, st:st + cw]
                    ckq.dma_start(t[:], src)
                    for bb in range(st // P, (st + cw) // P):
                        chunk_map[bb] = (t, bb * P - st)
                    pending.remove((st, cw))

            prefetch(0)  # chunk0 on its own queue, parallel with mt01
            if split_mt == "h":
                mtq.dma_start(mt01[:, :, 512:N], mat[:, 0:2, 512:N])
            mtq.dma_start(mt23[:], mat[:, 2:4, :])

            # PE warmup: dummy matmuls (no DMA deps) hold the HAM clock
            # hot while the first inputs stream in
            if warmup_mms:
                wfd = max(warmup_fd, P)
                scratch = cpool.tile([P, 2, wfd], fp8, tag="scratch")
                if not warmup_noinit:
                    nc.vector.memset(scratch[:], 0)
                wp = ppool.tile([P, N], f32, tag="ps", name="ps_warm")
                for _ in range(warmup_mms):
                    nc.tensor.matmul(wp[:, 0:wfd], scratch[:, :, 0:P],
                                     scratch[:], start=True, stop=True,
                                     perf_mode=DR, skip_group_check=True)

            for it in range(reps):
                for b in range(NB):
                    prefetch(b + pf)
                    ps = ppool.tile([P, N], f32, tag="ps", name="ps")
                    ut, boff = chunk_map[b]
                    for ks in range(0, KT, 2):
                        mtt, mks = mt_of[ks]
                        for h in range(2):
                            nc.tensor.matmul(
                                ps[:, h * 512:(h + 1) * 512],
                                ut[:, ks:ks + 2, boff:boff + P],
                                mtt[:, mks:mks + 2, h * 512:(h + 1) * 512],
                                start=(ks == 0),
                                stop=(ks == KT - 2),
                                perf_mode=DR,
                                skip_group_check=True,
                            )
                    t16 = wpool.tile([P, N], out_dt, tag="t16")
                    if evict_split == "tile" or b >= NB - fast_tail:
                        # eviction split across BOTH engines (different
                        # psum banks), one output DMA waiting on both
                        nc.scalar.activation(t16[:, 0:512], ps[:, 0:512],
                                             mybir.ActivationFunctionType.Copy)
                        nc.vector.tensor_copy(t16[:, 512:N], ps[:, 512:N])
                        oq = oq0 if b % 2 == 0 else oq1
                        oq.dma_start(y[b * P:(b + 1) * P, :], t16[:])
                        continue
                    if b % 2 == 0:
                        nc.scalar.activation(t16[:], ps[:],
                                             mybir.ActivationFunctionType.Copy)
                        oq0.dma_start(y[b * P:(b + 1) * P, :], t16[:])
                    else:
                        nc.vector.tensor_copy(t16[:], ps[:])
                        oq1.dma_start(y[b * P:(b + 1) * P, :], t16[:])
    nc.compile()
    return nc


def _build_nc_v6(reps=1, warm=(128, 256, 512), main_chunk=2048,
                 chunk_bufs=3, wbufs=4, warmup_mms=5, pf=12, out_u8=True,
                 warmup_fd=512, group=4, in_pkt=4096, dummy_dma=True,
                 tail_splits=(2048, 1024, 1024), psum_bufs=4,
                 warmup_noinit=False, pf_bottom=False,
                 gate_min_wi=99, gate_lead=7, chunk_lead=None):
    """v6: output DMA packet-size fix.

    Trace evidence: output DMAs ([128,1024] u8 -> 1KB DRAM rows) sustain
    only ~180GB/s (per-packet overhead ~45ns + 46ns transfer per 1KB), so
    the output stream (needs 148GB/s avg, bursts when inputs compete)
    backlogs and the drain runs ~8us past the last matmul. Fix: group
    `group` consecutive b-tiles into one DMA with DRAM layout
    [NB/group, P, group*1024] (partition p's rows from `group` tiles
    contiguous -> group-KB packets, ~300GB/s at 4KB). Host reassembles
    with a transpose (it already does &1). Input chunk packets capped at
    `in_pkt` bytes via AP grouping so round-robin stays ~fair.

    Tail taper: the last group's DMA is split by `tail_splits` (bytes of
    the group's 4096-col span per sub-DMA, last entries = the last
    tiles) so the final serial chain after the last matmul is short; the
    last two tiles' evictions split across ACT+DVE halves.
    """
    import concourse.tile as tile
    from concourse import bacc, mybir
    from concourse.tile_rust import add_dep_helper

    nc = bacc.Bacc("TRN2", target_bir_lowering=False, debug=False)
    fp8 = mybir.dt.float8e4
    f32 = mybir.dt.float32
    i16 = mybir.dt.int16
    DR = mybir.MatmulPerfMode.DoubleRow

    u8 = mybir.dt.uint8
    out_dt = u8 if out_u8 else i16
    GN = group * N          # output columns per group row
    NG = NB // group        # number of groups

    # chunk-major u layout (contiguous per chunk)
    uT = nc.declare_dram_parameter("uT", [P, KT * SHARD], fp8, isOutput=False)
    mat = nc.declare_dram_parameter("mat", [P, KT, N], fp8, isOutput=False)
    y = nc.declare_dram_parameter("y", [NG, P, GN], out_dt, isOutput=True)

    starts, chunks = chunk_schedule(warm, main_chunk)

    with tile.TileContext(nc) as tc:
        with (
            tc.tile_pool(name="consts", bufs=1) as cpool,
            tc.tile_pool(name="uchunks", bufs=chunk_bufs) as upool,
            tc.tile_pool(name="work", bufs=wbufs) as wpool,
            tc.tile_pool(name="psum", bufs=psum_bufs, space="PSUM") as ppool,
        ):
            if dummy_dma:
                wake = cpool.tile([1, 1], fp8, tag="wake")
                nc.sync.dma_start(wake[:], mat[0:1, 0:1, 0:1])
            mt01 = cpool.tile([P, 2, N], fp8, tag="mt01")
            mt23 = cpool.tile([P, 2, N], fp8, tag="mt23")
            nc.sync.dma_start(mt01[:, :, 0:512], mat[:, 0:2, 0:512])
            mt_of = {0: (mt01, 0), 2: (mt23, 0)}

            chunk_map = {}
            pending = list(zip(starts, chunks))
            last_mm = {}   # tile index -> last matmul instruction of tile
            warm_gate = [None]  # last warmup matmul

            def prefetch(upto_tile):
                for st, cw in pending[:]:
                    if st // P > upto_tile:
                        break
                    wi = starts.index(st)
                    pool = cpool if wi < len(warm) else upool
                    t = pool.tile([P, KT, cw], fp8,
                                  tag=("uw%d" % wi if wi < len(warm) else "uc"),
                                  name=f"uc{st}")
                    off = KT * st
                    src = uT[:, off:off + KT * cw].rearrange(
                        "p (k c) -> p k c", k=KT)
                    dma = nc.sync.dma_start(t[:], src)
                    # hold big chunks off the wire until the PE reaches a
                    # matmul ~gate_lead tiles before the chunk is needed:
                    # without this they hit the DMA engines immediately
                    # (8KB packets out-compete the 2KB mt transfers the
                    # first tiles gate on in the per-packet round-robin)
                    if wi >= gate_min_wi:
                        gt = st // P - gate_lead
                        gate = None
                        emitted = [bb for bb in last_mm if bb <= gt]
                        if emitted:
                            gate = last_mm[max(emitted)]
                        elif last_mm:
                            gate = last_mm[min(last_mm)]
                        else:
                            gate = warm_gate[0]
                        if gate is not None:
                            add_dep_helper(
                                dma.ins, gate.ins,
                                reason="hold chunk DMA until PE progress")
                    for bb in range(st // P, (st + cw) // P):
                        chunk_map[bb] = (t, bb * P - st)
                    pending.remove((st, cw))

            prefetch(0)
            nc.sync.dma_start(mt01[:, :, 512:N], mat[:, 0:2, 512:N])
            nc.sync.dma_start(mt23[:], mat[:, 2:4, :])
            if chunk_lead is not None:
                # emit the remaining warm chunks (ungated — needed at
                # tiles 1..warm_end and small enough not to hog the wire)
                prefetch(sum(warm) // P - 1)

            if warmup_mms:
                wfd = max(warmup_fd, P)
                scratch = cpool.tile([P, 2, wfd], fp8, tag="scratch")
                if not warmup_noinit:
                    nc.vector.memset(scratch[:], 0)
                wp = ppool.tile([P, N], f32, tag="ps", name="ps_warm")
                for _ in range(warmup_mms):
                    warm_gate[0] = nc.tensor.matmul(
                        wp[:, 0:wfd], scratch[:, :, 0:P],
                        scratch[:], start=True, stop=True,
                        perf_mode=DR, skip_group_check=True)

            # chunk_lead mode: main chunks (wi >= len(warm)) are emitted
            # right after the out-DMA of group (start_tile-chunk_lead)//
            # group, whose eviction wait blocks the SP queue head — this
            # holds the 8KB-packet chunk transfers off the wire (they
            # out-compete mt/warm loads in per-packet round-robin)
            # without any extra instructions or dependency surgery.
            chunk_after_group = {}
            if chunk_lead is not None:
                for st, cw in list(pending):
                    wi = starts.index(st)
                    if wi < len(warm):
                        continue
                    gk = max(0, (st // P - chunk_lead)) // group
                    chunk_after_group.setdefault(gk, []).append(st // P)

            for it in range(reps):
                tq = None
                for b in range(NB):
                    if not pf_bottom and chunk_lead is None:
                        prefetch(b + pf)
                    ps = ppool.tile([P, N], f32, tag="ps", name="ps")
                    ut, boff = chunk_map[b]
                    for ks in range(0, KT, 2):
                        mtt, mks = mt_of[ks]
                        for h in range(2):
                            last_mm[b] = nc.tensor.matmul(
                                ps[:, h * 512:(h + 1) * 512],
                                ut[:, ks:ks + 2, boff:boff + P],
                                mtt[:, mks:mks + 2, h * 512:(h + 1) * 512],
                                start=(ks == 0),
                                stop=(ks == KT - 2),
                                perf_mode=DR,
                                skip_group_check=True,
                            )
                    g, t = divmod(b, group)
                    if t == 0:
                        tq = wpool.tile([P, GN], out_dt, tag="tq")
                    dst_col = t * N
                    last_group = g == NG - 1
                    if last_group and t >= group - 2:
                        # final two tiles: halves on both engines
                        nc.scalar.activation(
                            tq[:, dst_col:dst_col + 512], ps[:, 0:512],
                            mybir.ActivationFunctionType.Copy)
                        nc.vector.tensor_copy(
                            tq[:, dst_col + 512:dst_col + N], ps[:, 512:N])
                    elif b % 2 == 0:
                        nc.scalar.activation(
                            tq[:, dst_col:dst_col + N], ps[:],
                            mybir.ActivationFunctionType.Copy)
                    else:
                        nc.vector.tensor_copy(
                            tq[:, dst_col:dst_col + N], ps[:])
                    if last_group and tail_splits:
                        # emit each sub-DMA right after the tile that
                        # completes its span, so issue overlaps the
                        # remaining matmuls and the final chain is short
                        ends, acc = [], GN - sum(tail_splits)
                        for w in tail_splits:
                            acc += w
                            ends.append(acc)
                        done_col = (t + 1) * N
                        col0 = GN - sum(tail_splits)
                        if t == 0 and col0:
                            pass  # head span handled when its end tile evicts
                        for i_s, e in enumerate(ends):
                            if e == done_col:
                                s = (ends[i_s - 1] if i_s else col0)
                                if i_s == 0 and col0:
                                    s = 0  # fold the head span into split 0
                                nc.sync.dma_start(y[g, :, s:e],
                                                  tq[:, s:e])
                    elif t == group - 1:
                        nc.sync.dma_start(y[g, :, :], tq[:])
                    if t == group - 1 and chunk_lead is not None:
                        for st_tile in chunk_after_group.get(g, []):
                            prefetch(st_tile)
                    if pf_bottom:
                        # emit chunk DMAs AFTER this tile's output DMA:
                        # the out-DMA's eviction-wait blocks the SP queue
                        # head, so a main chunk can't hit the wire early
                        # and steal engine bandwidth from mt/warm-chunk
                        # loads (queue FIFO orders starts, transfers
                        # overlap otherwise)
                        prefetch(b + 1 + pf)
    nc.compile()
    return nc


# v7 input blob layout (bytes per partition, in stream order):
# [c0 512 | mt01 2048 | mt23 2048 | c1 1024 | c2 2048 | c3 8192 |
#  c4 8192 | c5 8192 | c6 4608]  -> total 36864 = KT*SHARD + KT*N
V7_WARM = (128, 256, 512)           # c0..c2 batch widths
V7_MAIN = (2048, 2048, 2048, 1152)  # c3..c6
V7_OFF = {}
_o = 0
for _nm, _w in [("c0", 512), ("mt01", 2048), ("mt23", 2048),
                ("c1", 1024), ("c2", 2048), ("c3", 8192),
                ("c4", 8192), ("c5", 8192), ("c6", 4608)]:
    V7_OFF[_nm] = (_o, _o + _w)
    _o += _w
V7_TOTAL = _o


def _build_nc_v7(reps=1, wbufs=4, warmup_mms=20, warmup_fd=128, out_u8=True,
                 group=4, tail_splits=(2048, 1024, 1024), psum_bufs=4,
                 dummy_dma=True, chunk_bufs=2, out_q="sync",
                 chunk_after=(0, 1, 2), psum_half=False):
    """v7: single-FIFO-queue schedule built from measured DMA behavior.

    Measured: DMAs on one queue transfer strictly FIFO (no bandwidth
    stealing), but each DMA instruction costs ~0.5-0.6us of dead handoff
    before its packets flow. So the early loads are packed into a host-
    side contiguous blob and fetched as 3 big DMAs ([c0|mt01], [mt23],
    [c1|c2]) instead of 6 small ones, and the main chunks are emitted
    between output-group DMAs so the FIFO position (not semaphores)
    paces them. Outputs grouped `group` tiles per DMA (4KB packets,
    ~300GB/s vs ~180 at 1KB). PE warmup uses a small scratch (short
    memset -> warmups from ~7.4us) so the clock ramp finishes early.
    """
    import concourse.tile as tile
    from concourse import bacc, mybir

    nc = bacc.Bacc("TRN2", target_bir_lowering=False, debug=False)
    fp8 = mybir.dt.float8e4
    f32 = mybir.dt.float32
    i16 = mybir.dt.int16
    DR = mybir.MatmulPerfMode.DoubleRow

    u8 = mybir.dt.uint8
    out_dt = u8 if out_u8 else i16
    GN = group * N
    NG = NB // group

    blob = nc.declare_dram_parameter("blob", [P, V7_TOTAL], fp8,
                                     isOutput=False)
    y = nc.declare_dram_parameter("y", [NG, P, GN], out_dt, isOutput=True)

    warm_starts = [0, 128, 384]
    main_starts = [896, 2944, 4992, 7040]

    with tile.TileContext(nc) as tc:
        oq = {"sync": nc.sync, "scalar": nc.scalar,
              "gpsimd": nc.gpsimd}[out_q]
        with (
            tc.tile_pool(name="consts", bufs=1) as cpool,
            tc.tile_pool(name="uchunks", bufs=chunk_bufs) as upool,
            tc.tile_pool(name="work", bufs=wbufs) as wpool,
            tc.tile_pool(name="psum", bufs=psum_bufs, space="PSUM") as ppool,
        ):
            if dummy_dma:
                wake = cpool.tile([1, 1], fp8, tag="wake")
                nc.sync.dma_start(wake[:], blob[0:1, 0:1])
            big1 = cpool.tile([P, 4608], fp8, tag="big1")
            w12 = cpool.tile([P, 3072], fp8, tag="w12")
            # one DMA for everything tile 0 needs (one handoff, one sem);
            # c1 and c2 separate so tiles 1-6 unblock incrementally
            nc.sync.dma_start(big1[:], blob[:, 0:4608])
            nc.sync.dma_start(w12[:, 0:1024], blob[:, 4608:5632])
            nc.sync.dma_start(w12[:, 1024:3072], blob[:, 5632:7680])

            mt01v = big1[:, 512:2560].rearrange("p (k n) -> p k n", k=2)
            mt23v = big1[:, 2560:4608].rearrange("p (k n) -> p k n", k=2)
            mt_of = {0: mt01v, 2: mt23v}

            # chunk views for warm tiles
            chunk_map = {}
            c0v = big1[:, 0:512].rearrange("p (k c) -> p k c", k=KT)
            chunk_map[0] = (c0v, 0)
            c1v = w12[:, 0:1024].rearrange("p (k c) -> p k c", k=KT)
            for b in (1, 2):
                chunk_map[b] = (c1v, b * P - 128)
            c2v = w12[:, 1024:3072].rearrange("p (k c) -> p k c", k=KT)
            for b in range(3, 7):
                chunk_map[b] = (c2v, b * P - 384)

            main_pending = []
            for ci, st in enumerate(main_starts):
                nm = f"c{ci + 3}"
                o0, o1 = V7_OFF[nm]
                cw = (o1 - o0) // KT
                main_pending.append((st, cw, o0, o1, nm))

            def emit_main():
                st, cw, o0, o1, nm = main_pending.pop(0)
                t = upool.tile([P, KT, cw], fp8, tag="uc", name=nm)
                nc.sync.dma_start(
                    t[:], blob[:, o0:o1].rearrange("p (k c) -> p k c", k=KT))
                for bb in range(st // P, st // P + cw // P):
                    chunk_map[bb] = (t, bb * P - st)

            emit_main()  # c3 right behind the warm loads in FIFO

            if warmup_mms:
                wfd = max(warmup_fd, P)
                scratch = cpool.tile([P, 2, wfd], fp8, tag="scratch")
                nc.vector.memset(scratch[:], 0)
                wp = ppool.tile([P, 512 if psum_half else N], f32,
                                tag="ps", name="ps_warm")
                for _ in range(warmup_mms):
                    nc.tensor.matmul(wp[:, 0:wfd], scratch[:, :, 0:P],
                                     scratch[:, :, 0:wfd], start=True,
                                     stop=True,
                                     perf_mode=DR, skip_group_check=True)

            for it in range(reps):
                tq = None
                for b in range(NB):
                    g, t = divmod(b, group)
                    if t == 0:
                        tq = wpool.tile([P, GN], out_dt, tag="tq")
                    dst_col = t * N
                    ut, boff = chunk_map[b]
                    if psum_half:
                        # per-half psum tiles (1 bank, 8 bufs): finer
                        # release granularity; h0 evicts on ACT, h1 on
                        # DVE every tile (~620ns each per 864ns cadence).
                        # ks-outer order: mt23 is only needed at MM3, so
                        # tile 0 starts as soon as [c0|mt01] lands.
                        pshs = [ppool.tile([P, 512], f32, tag="ps",
                                           name="ps") for _ in range(2)]
                        for ks in range(0, KT, 2):
                            for h in range(2):
                                nc.tensor.matmul(
                                    pshs[h][:],
                                    ut[:, ks:ks + 2, boff:boff + P],
                                    mt_of[ks][:, 0:2,
                                              h * 512:(h + 1) * 512],
                                    start=(ks == 0),
                                    stop=(ks == KT - 2),
                                    perf_mode=DR,
                                    skip_group_check=True,
                                )
                        nc.scalar.activation(
                            tq[:, dst_col:dst_col + 512], pshs[0][:],
                            mybir.ActivationFunctionType.Copy)
                        nc.vector.tensor_copy(
                            tq[:, dst_col + 512:dst_col + N], pshs[1][:])
                        last_group = g == NG - 1
                    else:
                        ps = ppool.tile([P, N], f32, tag="ps", name="ps")
                        for ks in range(0, KT, 2):
                            mtt = mt_of[ks]
                            for h in range(2):
                                nc.tensor.matmul(
                                    ps[:, h * 512:(h + 1) * 512],
                                    ut[:, ks:ks + 2, boff:boff + P],
                                    mtt[:, 0:2, h * 512:(h + 1) * 512],
                                    start=(ks == 0),
                                    stop=(ks == KT - 2),
                                    perf_mode=DR,
                                    skip_group_check=True,
                                )
                        last_group = g == NG - 1
                        if last_group and t >= group - 2:
                            nc.scalar.activation(
                                tq[:, dst_col:dst_col + 512], ps[:, 0:512],
                                mybir.ActivationFunctionType.Copy)
                            nc.vector.tensor_copy(
                                tq[:, dst_col + 512:dst_col + N],
                                ps[:, 512:N])
                        elif b % 2 == 0:
                            nc.scalar.activation(
                                tq[:, dst_col:dst_col + N], ps[:],
                                mybir.ActivationFunctionType.Copy)
                        else:
                            nc.vector.tensor_copy(
                                tq[:, dst_col:dst_col + N], ps[:])
                    if last_group and tail_splits:
                        ends, acc = [], GN - sum(tail_splits)
                        for w in tail_splits:
                            acc += w
                            ends.append(acc)
                        done_col = (t + 1) * N
                        col0 = GN - sum(tail_splits)
                        for i_s, e in enumerate(ends):
                            if done_col - N < e <= done_col:
                                s = (ends[i_s - 1] if i_s else col0)
                                if i_s == 0 and col0:
                                    s = 0
                                nc.sync.dma_start(y[g, :, s:e], tq[:, s:e])
                    elif t == group - 1:
                        oq.dma_start(y[g, :, :], tq[:])
                        if g in chunk_after and main_pending:
                            emit_main()
    nc.compile()
    return nc


EVICT = "i16out"
W1_ACT = 48      # 48/64 PSUM->i16 converts on ACT, 16/64 on DVE
U_CHUNKS = 8     # input u loaded in 8 chunks so matmuls start early
WBUFS = 6
KS_OUTER = True  # k-pair outer loop: one LDWEIGHTS serves both psum halves

VERSION = 54
V7_OPTS_A = dict(wbufs=4, warmup_mms=20, warmup_fd=128, group=4,
                 tail_splits=(2048, 1024, 1024), dummy_dma=True,
                 chunk_bufs=2, out_q="sync", chunk_after=(0, 1, 2))
V7_OPTS_B = dict(wbufs=4, warmup_mms=28, warmup_fd=128, group=4,
                 tail_splits=(2048, 1024, 1024), dummy_dma=False,
                 chunk_bufs=2, out_q="sync", chunk_after=(0, 1, 2),
                 psum_half=True, psum_bufs=8)
V7_OPTS_C = dict(wbufs=4, warmup_mms=28, warmup_fd=128, group=4,
                 tail_splits=(2048, 1024, 512, 512), dummy_dma=False,
                 chunk_bufs=2, out_q="sync", chunk_after=(0, 1, 2),
                 psum_half=True, psum_bufs=8)
V7_OPTS_D = dict(wbufs=4, warmup_mms=31, warmup_fd=128, group=4,
                 tail_splits=(2048, 1024, 1024), dummy_dma=False,
                 chunk_bufs=2, out_q="sync", chunk_after=(0, 1, 2),
                 psum_half=True, psum_bufs=8)
V7_OPTS_E = dict(wbufs=4, warmup_mms=28, warmup_fd=128, group=4,
                 tail_splits=(2048, 1024, 1024), dummy_dma=True,
                 chunk_bufs=2, out_q="sync", chunk_after=(0, 1, 2),
                 psum_half=True, psum_bufs=8)
V6_OPTS_A = dict(warm=(128, 256, 512), main_chunk=2048,
                 chunk_bufs=3, wbufs=4, warmup_mms=5, pf=12,
                 group=4, in_pkt=4096, dummy_dma=True,
                 tail_splits=(2048, 1024, 1024))
# v6b: chunk DMAs gated on PE progress (explicit deps) + uninterrupted
# warmups from ~7.2us (no memset gate) so the PE clock ramp completes
V6_OPTS_B = dict(warm=(128, 256, 512), main_chunk=2048,
                 chunk_bufs=3, wbufs=4, warmup_mms=6, pf=12,
                 group=4, in_pkt=4096, dummy_dma=True,
                 tail_splits=(2048, 1024, 1024),
                 chunk_lead=12)
V4_OPTS_A = dict(warm=(128, 256, 512), main_chunk=2048,
                 chunk_bufs=3, wbufs=8, warmup_mms=5, pf=12,
                 chunk_major=True, fast_tail=2)
# v5: single SP queue (extra HWDGE queues cost ~1.1us teardown each),
# chunk-major packets, ring-wake dummy DMA, per-tile split evictions
V5_OPTS_A = dict(warm=(128, 256, 512), main_chunk=2048,
                 chunk_bufs=3, wbufs=8, warmup_mms=6, pf=12,
                 chunk_major=True, fast_tail=0, dummy_dma=True,
                 evict_split="tile", mt_q="sync", chunk_q="sync",
                 out_q=("sync", "sync"))
V2_OPTS = dict(act_pairs=17, warm=(256, 256, 512), main_chunk=1024,
               chunk_bufs=3, wbufs=3, mt_splits=1)
V3_OPTS = dict(warm=(256, 256, 512), main_chunk=1024,
               chunk_bufs=3, wbufs=6, warmup_mms=12, pf=8)
V3_OPTS_B = dict(warm=(128, 256, 512, 1024), main_chunk=1024,
                 chunk_bufs=3, wbufs=8, warmup_mms=22, pf=8)
V3_OPTS_C = dict(warm=(128, 256, 512), main_chunk=2048,
                 chunk_bufs=3, wbufs=8, warmup_mms=10, pf=12)
V3_OPTS_D = dict(warm=(128, 256, 512), main_chunk=2048,
                 chunk_bufs=3, wbufs=8, warmup_mms=6, pf=12)
V3_OPTS_E = dict(warm=(128, 256, 512), main_chunk=2048,
                 chunk_bufs=3, wbufs=4, warmup_mms=10, pf=12, pair_dma=True)
V3_OPTS_F = dict(warm=(128, 256, 512), main_chunk=2048,
                 chunk_bufs=3, wbufs=8, warmup_mms=10, pf=12,
                 chunk_major=True)
V3_OPTS_G = dict(warm=(128, 256, 512), main_chunk=2048,
                 chunk_bufs=3, wbufs=8, warmup_mms=12, pf=12, warmup_fd=128)
V3_OPTS_H = dict(warm=(128, 256, 512), main_chunk=2048,
                 chunk_bufs=3, wbufs=8, warmup_mms=18, pf=12, warmup_fd=128)
V3_OPTS_I = dict(warm=(128, 256, 512), main_chunk=2048,
                 chunk_bufs=3, wbufs=8, warmup_mms=8, pf=12, split_mt=True)
V3_OPTS_J = dict(warm=(128, 256, 512), main_chunk=2048,
                 chunk_bufs=3, wbufs=8, warmup_mms=5, pf=4, split_mt="fine")
V3_OPTS_K = dict(warm=(128, 256, 512), main_chunk=2048,
                 chunk_bufs=3, wbufs=8, warmup_mms=8, pf=12, split_mt=True,
                 alt_out_queue=True)
V3_OPTS_L = dict(warm=(128, 256, 512), main_chunk=2048,
                 chunk_bufs=4, wbufs=8, warmup_mms=7, pf=6, split_mt=True)
V3_OPTS_M = dict(warm=(128, 256, 512), main_chunk=2048,
                 chunk_bufs=3, wbufs=8, warmup_mms=8, pf=12, split_mt=True,
                 fast_tail=2)
V3_OPTS_N = dict(warm=(128, 256, 512), main_chunk=2048,
                 chunk_bufs=3, wbufs=8, warmup_mms=7, pf=4, split_mt=True)
V3_OPTS_P = dict(warm=(128, 256, 512), main_chunk=2048,
                 chunk_bufs=3, wbufs=8, warmup_mms=11, pf=12, split_mt="h",
                 warmup_noinit=True)
V3_OPTS_Q = dict(warm=(128, 256, 512), main_chunk=2048,
                 chunk_bufs=3, wbufs=8, warmup_mms=6, pf=12, split_mt=True)
V3_OPTS_R = dict(warm=(128, 256, 512), main_chunk=2048,
                 chunk_bufs=3, wbufs=8, warmup_mms=4, pf=12, split_mt=True)
V3_OPTS_S = dict(warm=(128, 256, 512), main_chunk=2048,
                 chunk_bufs=3, wbufs=8, warmup_mms=6, pf=12, split_mt="h")
V3_OPTS_T = dict(warm=(128, 256, 512), main_chunk=2048,
                 chunk_bufs=3, wbufs=8, warmup_mms=5, pf=12, split_mt="h")


def _active_opts():
    return {5: V3_OPTS_C, 6: V3_OPTS_D, 7: V3_OPTS_E, 8: V3_OPTS_F,
            9: V3_OPTS_G, 10: V3_OPTS_H, 11: V3_OPTS_I, 12: V3_OPTS_J,
            13: V3_OPTS_K, 14: V3_OPTS_L, 15: V3_OPTS_M, 16: V3_OPTS_N,
            17: V3_OPTS_P, 18: V3_OPTS_Q, 19: V3_OPTS_R, 20: V3_OPTS_S,
            21: V3_OPTS_T, 30: V4_OPTS_A, 31: V5_OPTS_A, 40: V6_OPTS_A,
            41: V6_OPTS_B, 50: V7_OPTS_A, 51: V7_OPTS_B, 52: V7_OPTS_C,
            53: V7_OPTS_D, 54: V7_OPTS_E,
            4: V3_OPTS_B, 3: V3_OPTS}.get(VERSION, V3_OPTS_C)


def get_nc(reps=1, out_u8=True):
    if VERSION >= 50:
        opts = _active_opts()
        key = (VERSION, reps, tuple(sorted(opts.items())), out_u8)
        if key not in _nc_cache:
            _nc_cache[key] = _build_nc_v7(reps, out_u8=out_u8, **opts)
        return _nc_cache[key]
    if VERSION >= 40:
        opts = _active_opts()
        key = (VERSION, reps, tuple(sorted(opts.items())), out_u8)
        if key not in _nc_cache:
            _nc_cache[key] = _build_nc_v6(reps, out_u8=out_u8, **opts)
        return _nc_cache[key]
    if VERSION >= 30:
        opts = _active_opts()
        key = (VERSION, reps, tuple(sorted(opts.items())), out_u8)
        if key not in _nc_cache:
            _nc_cache[key] = _build_nc_v4(reps, out_u8=out_u8, **opts)
        return _nc_cache[key]
    if VERSION in (9, 10, 11, 12, 13, 14, 15, 16, 17, 18, 19, 20, 21):
        opts = _active_opts()
        key = (VERSION, reps, tuple(sorted(opts.items())), out_u8)
        if key not in _nc_cache:
            _nc_cache[key] = _build_nc_v3(reps, out_u8=out_u8, **opts)
        return _nc_cache[key]
    if VERSION == 8:
        key = (8, reps, tuple(sorted(V3_OPTS_F.items())), out_u8)
        if key not in _nc_cache:
            _nc_cache[key] = _build_nc_v3(reps, out_u8=out_u8, **V3_OPTS_F)
        return _nc_cache[key]
    if VERSION == 7:
        key = (7, reps, tuple(sorted(V3_OPTS_E.items())), out_u8)
        if key not in _nc_cache:
            _nc_cache[key] = _build_nc_v3(reps, out_u8=out_u8, **V3_OPTS_E)
        return _nc_cache[key]
    if VERSION == 6:
        key = (6, reps, tuple(sorted(V3_OPTS_D.items())), out_u8)
        if key not in _nc_cache:
            _nc_cache[key] = _build_nc_v3(reps, out_u8=out_u8, **V3_OPTS_D)
        return _nc_cache[key]
    if VERSION == 5:
        key = (5, reps, tuple(sorted(V3_OPTS_C.items())), out_u8)
        if key not in _nc_cache:
            _nc_cache[key] = _build_nc_v3(reps, out_u8=out_u8, **V3_OPTS_C)
        return _nc_cache[key]
    if VERSION == 4:
        key = (4, reps, tuple(sorted(V3_OPTS_B.items())), out_u8)
        if key not in _nc_cache:
            _nc_cache[key] = _build_nc_v3(reps, out_u8=out_u8, **V3_OPTS_B)
        return _nc_cache[key]
    if VERSION == 3:
        key = (3, reps, tuple(sorted(V3_OPTS.items())), out_u8)
        if key not in _nc_cache:
            _nc_cache[key] = _build_nc_v3(reps, out_u8=out_u8, **V3_OPTS)
        return _nc_cache[key]
    if VERSION == 2:
        key = (2, reps, tuple(sorted(V2_OPTS.items())))
        if key not in _nc_cache:
            _nc_cache[key] = _build_nc_v2(reps, **V2_OPTS)
        return _nc_cache[key]
    key = (reps, EVICT, W1_ACT, U_CHUNKS, WBUFS, KS_OUTER)
    if key not in _nc_cache:
        _nc_cache[key] = _build_nc(reps, evict=EVICT, w1_act=W1_ACT,
                                   u_chunks=U_CHUNKS, wbufs=WBUFS,
                                   ks_outer=KS_OUTER)
    return _nc_cache[key]


def _to_k_major(a_km, free):
    """[K, free] -> [P, KT, free] with k = ks*128 + p."""
    return np.ascontiguousarray(
        a_km.reshape(KT, P, free).transpose(1, 0, 2)
    )


def make_in_maps(u, M):
    u8 = np.asarray(u).astype(FP8_NP)
    m8 = np.asarray(M).astype(FP8_NP)
    mat3 = _to_k_major(m8, N)
    if VERSION >= 50:
        # v7 blob: [c0 | mt01 | mt23 | c1 | c2 | c3 | c4 | c5 | c6]
        starts = [0, 128, 384, 896, 2944, 4992, 7040]
        widths = [128, 256, 512, 2048, 2048, 2048, 1152]
        mt01 = mat3[:, 0:2, :].reshape(P, 2 * N)
        mt23 = mat3[:, 2:4, :].reshape(P, 2 * N)
        in_maps = []
        for i in range(N_CORES):
            uT_i = np.ascontiguousarray(u8[i * SHARD:(i + 1) * SHARD, :].T)
            uk = _to_k_major(uT_i, SHARD)
            ch = [uk[:, :, st:st + cw].reshape(P, KT * cw)
                  for st, cw in zip(starts, widths)]
            blob = np.concatenate(
                [ch[0], mt01, mt23, ch[1], ch[2], ch[3], ch[4], ch[5],
                 ch[6]], axis=1)
            assert blob.shape == (P, V7_TOTAL)
            in_maps.append({"blob": np.ascontiguousarray(blob)})
        return in_maps
    opts = _active_opts() if VERSION >= 3 else {}
    chunk_major = bool(opts.get("chunk_major")) or VERSION >= 40
    if chunk_major:
        starts, chunks = chunk_schedule(opts["warm"], opts["main_chunk"])
    in_maps = []
    for i in range(N_CORES):
        uT_i = np.ascontiguousarray(u8[i * SHARD:(i + 1) * SHARD, :].T)
        uk = _to_k_major(uT_i, SHARD)  # [P, KT, SHARD]
        if chunk_major:
            uk = np.concatenate(
                [uk[:, :, st:st + cw].reshape(P, KT * cw)
                 for st, cw in zip(starts, chunks)], axis=1)
        in_maps.append({"uT": uk, "mat": mat3})
    return in_maps


def ungroup_y(yc, group):
    """[NB/group, P, group*N] grouped layout -> [SHARD, N]."""
    ng = NB // group
    return np.ascontiguousarray(
        yc.reshape(ng, P, group, N).transpose(0, 2, 1, 3).reshape(SHARD, N))


def kernel(u, crc_gen, info_pos, ind_gather, perm_out):
    from concourse.bass_utils import run_bass_kernel_spmd

    M = build_M(crc_gen, info_pos, ind_gather, perm_out)
    in_maps = make_in_maps(u, M)
    nc = get_nc()
    res = run_bass_kernel_spmd(nc, in_maps, core_ids=list(range(N_CORES)))
    ys = [np.asarray(r["y"]) for r in res.results]
    if ys[0].dtype == np.uint8 and any((yc == 255).any() for yc in ys):
        # saturation certificate failed (a sum may have clipped at 255):
        # rerun with exact i16 output
        nc16 = get_nc(out_u8=False)
        res = run_bass_kernel_spmd(nc16, in_maps,
                                   core_ids=list(range(N_CORES)))
        ys = [np.asarray(r["y"]) for r in res.results]
    group = _active_opts().get("group") if VERSION >= 40 else None
    if group:
        ys = [ungroup_y(yc, group) for yc in ys]
    out = np.concatenate([(yc & 1).astype(np.float32) for yc in ys], axis=0)
    return out



# revision 55
# speedup vs baseline: 1.0325x; 1.0325x over previous
"""5G Polar encoder (CRC11 + subchannel alloc + butterfly + interleave) on 8 trn2 cores.

The whole reference computation is GF(2)-linear in u:
    parity  = (u @ crc_gen) mod 2                       -> linear
    bits    = [u | parity] = u @ [I | crc_gen]          -> linear
    scatter x[:, info_pos] = bits                       -> column selection (linear)
    butterfly stages x ^= x[:, g[s]]                    -> linear over GF(2)
    out     = x[:, perm_out]                            -> column gather (linear)

So on the host we compose one binary matrix M [512, 1024] from the tiny index
tables (cheap uint8 ops), and the device kernel is a single fused
    y = (u @ M) mod 2
data-parallel over the batch: each of the 8 cores computes an [8192, 512] @
[512, 1024] matmul in fp8e4 with DoubleRow perf mode (exact: all values are
0/1, sums <= 523 accumulate in f32 PSUM).

Active design (VERSION=51 = _build_nc_v7 + V7_OPTS_B, ~72.0-72.6us/core
vs 74.1us for the previous v3/VERSION=20 design; NTFF min-of-3; rel err 0):
  - exec_time window = [first const MEMSET (~5.9us, framework preamble)
    .. end of the walrus-emitted teardown]. The teardown (zeroes all 255
    HW semaphores across 5 engines + barrier rounds) is ~6.7-8.7us and
    NOT controllable from kernel code; a trivial 1-copy kernel measures
    ~13us. Budget: ~5.4 front + 55.2 MM phase + ~2.2 dither + 2.6 tail
    + ~1.1 waits + ~7.6 teardown.
  - Input = ONE host-packed blob [128, 36864B] per core:
    [c0|mt01|mt23|c1|c2|c3|c4|c5|c6]. [c0|mt01|mt23] fetched as a single
    583KB DMA (one handoff, one sem -> tile0 fully ready ~10.8us), then
    c1, c2, c3..c6 as separate DMAs, all on the single SP queue. Within
    a queue transfers are strictly FIFO (no bandwidth stealing) but
    EVERY DMA instruction costs ~0.5-1.7us of dead handoff before its
    packets flow -> merge small early loads; 26 DMA instrs (v3 had 74).
  - Outputs grouped 4 b-tiles per DMA with DRAM layout [16,128,4096]
    (partition rows of 4 tiles contiguous -> 4KB packets ~300GB/s; 1KB
    row packets only sustain ~180GB/s, which backlogged v3's out queue).
    Host un-groups with a cheap transpose in kernel(). Last group's DMA
    split (2048,1024,1024) cols, each span emitted as its tiles evict;
    last two tiles' evictions split ACT/DVE halves.
  - PSUM halves [128,512] f32, bufs=8 (1 bank each): finer release
    granularity; ks-outer MM order (mt23 first needed at MM3); h0 half
    always evicts on ACT, h1 on DVE (~620ns each per 864ns tile). Raw
    sums out as u8 (saturation certificate: no 255 => exact, else rerun
    i16 build); host does &1.
  - 28 contiguous warmup MMs (fd=128, ~120ns each) from ~7.2us: the PE
    clock ramps to 2.4GHz only after ~3.5us of CONTIGUOUS PE activity
    (idle gaps reset it; cold MMs run 427ns vs 216 warm). Small scratch
    [128,2,128] memset (~300ns) so warmups start early.

Hard-won HW facts (measured on this machine; keep for future sessions):
  - fp8 DoubleRow 216ns/MM (N=512) is the PE floor: 157 TF/s cap. All
    Double* perf modes cap at 2x; no fp4/quad mode exists. Butterfly/
    Kronecker decompositions don't beat the dense GEMM: PE cycles =
    K-granules(256) x N-columns and rank(M)=512 forces 2 granules.
  - A fixed ~432ns PE stall recurs every 10.791us in EVERY build
    (clock-management dither, unavoidable, ~2.2us per run).
  - add_dep_helper(dma_inst, mm_inst) deps flipped the whole core into
    a 2.0GHz state (ALL engines 1.2x slower, 3/3 runs) - do not use.
  - Multi-queue DMA (scalar/gpsimd HWDGE): each extra queue adds ~1.1us
    teardown, and queues compete per-packet round-robin (big packets
    win, no prioritization) - single SP queue + FIFO order is better.
  - The Tile scheduler reorders same-queue DMAs that have no deps;
    emission order does NOT pin issue order.
  - tensor_scalar `mod` and ACT `Sin` don't work on HW; Pool (gpsimd)
    copies are ~4.25us/tile; fused AND+cast rejected ("TSP bitVec op
    cannot do cast"); warmup-on-uninitialized-SBUF rejected by Tile
    ("Releasing unallocated Tile ... read but not written").
"""

import numpy as np
import ml_dtypes

N_CORES = 8
BS = 65536
K = 512          # u feature dim (contraction)
N = 1024         # output columns
SHARD = BS // N_CORES  # 8192 batch rows per core
P = 128
KT = K // P      # 4 k-tiles
NB = SHARD // P  # 64 batch tiles per core

FP8_NP = ml_dtypes.float8_e4m3

_nc_cache = {}


def build_M(crc_gen, info_pos, ind_gather, perm_out):
    """Compose the encoder into one GF(2) matrix M [K, N]: out = (u @ M) mod 2."""
    crc_gen = np.asarray(crc_gen)
    info_pos = np.asarray(info_pos)
    ind_gather = np.asarray(ind_gather)
    perm_out = np.asarray(perm_out)
    k, _ = crc_gen.shape
    nb, n1 = ind_gather.shape
    kp = info_pos.shape[0]
    C = (crc_gen.astype(np.int64) & 1).astype(np.uint8)
    B = np.concatenate([np.eye(k, dtype=np.uint8), C], axis=1)  # [k, kp]
    # scatter bits into columns; duplicate indices: last write wins (matches
    # jax/numpy .at[].set application order)
    col_src = np.full(n1, -1, np.int64)
    col_src[info_pos] = np.arange(kp)
    A = np.zeros((k, n1), np.uint8)
    valid = col_src >= 0
    A[:, valid] = B[:, col_src[valid]]
    for s in range(nb):
        A = A ^ A[:, ind_gather[s]]
    return A[:, perm_out]  # [k, n]


def _build_nc(reps=1, do_mm=True, do_evict=True, evict="pool",
              w1_act=64, w3_dve=0, ev_stage=3, u_chunks=1, wbufs=4,
              ks_outer=False):
    """evict modes:
    - "pool":    ACT f32->i16, DVE AND, Pool narrow i16->i8, DMA i8
    - "dve":     ACT f32->i16, DVE AND, DVE narrow i16->i8, DMA i8
    - "i16out":  ACT f32->i16, DVE AND, DMA out i16 (host takes low bits)
    - "dmacast": ACT f32->i16, DVE AND, gpsimd casting DMA i16->i8
    - "split":   W1 on ACT for w1_act tiles/64 else DVE; AND on DVE;
                 narrow on DVE for w3_dve tiles/64 else Pool; DMA i8
    """
    import concourse.tile as tile
    from concourse import bacc, mybir

    nc = bacc.Bacc("TRN2", target_bir_lowering=False, debug=False)
    fp8 = mybir.dt.float8e4
    f32 = mybir.dt.float32
    i16 = mybir.dt.int16
    i8 = mybir.dt.int8
    DR = mybir.MatmulPerfMode.DoubleRow

    # k-major 3D layouts: [p, ks, free] with global k = ks*128 + p (both
    # operands use the same mapping, so the contraction is correct).
    uT = nc.declare_dram_parameter("uT", [P, KT, SHARD], fp8, isOutput=False)
    mat = nc.declare_dram_parameter("mat", [P, KT, N], fp8, isOutput=False)
    y_dt = i16 if evict == "i16out" else i8
    y = nc.declare_dram_parameter("y", [SHARD, N], y_dt, isOutput=True)

    with tile.TileContext(nc) as tc:
        with (
            tc.tile_pool(name="consts", bufs=1) as cpool,
            tc.tile_pool(name="work", bufs=wbufs) as wpool,
            tc.tile_pool(name="outs", bufs=4) as opool,
            tc.tile_pool(name="psum", bufs=4, space="PSUM") as ppool,
        ):
            mt = cpool.tile([P, KT, N], fp8, tag="mt")
            nc.sync.dma_start(mt[:], mat[:])
            # chunk the big u load along batch so the first b-tile's matmuls
            # start after ~1/u_chunks of the 4MB has landed
            CW = SHARD // u_chunks
            uts = []
            for c in range(u_chunks):
                ut_c = cpool.tile([P, KT, CW], fp8, tag=f"ut{c}", name=f"ut{c}")
                nc.sync.dma_start(ut_c[:], uT[:, :, c * CW:(c + 1) * CW])
                uts.append(ut_c)
            ot_shared = None
            if evict == "outonly":
                ot_shared = cpool.tile([P, N], i8, tag="ot_shared")
                nc.any.memset(ot_shared[:], 0)
            ps_shared = None
            if not do_mm:
                ps_shared = ppool.tile([P, N], f32, tag="ps_shared")
                for h in range(2):
                    nc.tensor.matmul(
                        ps_shared[:, h * 512:(h + 1) * 512],
                        uts[0][:, 0:2, 0:P],
                        mt[:, 0:2, h * 512:(h + 1) * 512],
                        start=True, stop=True, perf_mode=DR,
                    )
            for i, b in enumerate(
                [b for _ in range(reps) for b in range(NB)]
            ):
                if do_mm:
                    ps = ppool.tile([P, N], f32, tag="ps", name="ps")
                else:
                    ps = ps_shared
                t16 = wpool.tile([P, N], i16, tag="t16")
                a16 = wpool.tile([P, N], i16, tag="a16")
                ot = opool.tile([P, N], i8, tag="ot")
                if do_mm:
                    ut = uts[(b * P) // CW]
                    boff = (b * P) % CW
                    loop = (
                        [(h, ks) for ks in range(0, KT, 2) for h in range(2)]
                        if ks_outer else
                        [(h, ks) for h in range(2) for ks in range(0, KT, 2)]
                    )
                    for h, ks in loop:
                        nc.tensor.matmul(
                            ps[:, h * 512:(h + 1) * 512],
                            ut[:, ks:ks + 2, boff:boff + P],
                            mt[:, ks:ks + 2, h * 512:(h + 1) * 512],
                            start=(ks == 0),
                            stop=(ks == KT - 2),
                            perf_mode=DR,
                            skip_group_check=ks_outer,
                        )
                if do_evict:
                    if evict == "outonly":
                        nc.sync.dma_start(y[b * P:(b + 1) * P, :], ot_shared[:])
                        continue
                    # W1: PSUM f32 -> i16
                    if ev_stage >= 1:
                        if evict == "w1dve" or (i % NB) >= w1_act:
                            nc.vector.tensor_copy(t16[:], ps[:])
                        else:
                            nc.scalar.activation(
                                t16[:], ps[:],
                                mybir.ActivationFunctionType.Copy,
                            )
                    # W2: AND with 1
                    if ev_stage >= 2:
                        nc.vector.tensor_scalar(
                            a16[:], t16[:], 1, None,
                            mybir.AluOpType.bitwise_and,
                        )
                    # W3 + output DMA
                    if ev_stage < 3:
                        continue
                    if evict == "i16out":
                        nc.sync.dma_start(y[b * P:(b + 1) * P, :], a16[:])
                    elif evict in ("dmacast", "w1dve"):
                        nc.gpsimd.dma_start(y[b * P:(b + 1) * P, :], a16[:])
                    else:
                        if evict == "dve" or (
                            evict == "split" and (i % NB) < w3_dve
                        ):
                            nc.vector.tensor_copy(ot[:], a16[:])
                        else:
                            nc.gpsimd.tensor_copy(ot[:], a16[:])
                        nc.sync.dma_start(y[b * P:(b + 1) * P, :], ot[:])
    nc.compile()
    return nc


def _build_nc_v2(reps=1, act_pairs=22, warm=(256, 256), main_chunk=1024,
                 chunk_bufs=3, wbufs=3, mt_splits=4):
    """v2: pair eviction ([128,2048] f32 = 4 PSUM banks per evict instr),
    i8 output, W1 split ACT/DVE, staged input DMA with pool backpressure.

    Per pair (2 b-tiles): 8 matmuls fill 4 banks; one W1 (PSUM f32->i16,
    ACT for act_pairs/32 of pairs else DVE), one DVE AND (i16), one DVE
    narrow (i16->i8, safe post-AND), 2 output DMAs.
    """
    import concourse.tile as tile
    from concourse import bacc, mybir

    nc = bacc.Bacc("TRN2", target_bir_lowering=False, debug=False)
    fp8 = mybir.dt.float8e4
    f32 = mybir.dt.float32
    i16 = mybir.dt.int16
    i8 = mybir.dt.int8
    DR = mybir.MatmulPerfMode.DoubleRow

    uT = nc.declare_dram_parameter("uT", [P, KT, SHARD], fp8, isOutput=False)
    mat = nc.declare_dram_parameter("mat", [P, KT, N], fp8, isOutput=False)
    # raw i16 sums; host computes & 1
    y = nc.declare_dram_parameter("y", [SHARD, N], i16, isOutput=True)

    # batch chunk schedule: warmup chunks then fixed-size main chunks
    chunks = list(warm)
    while sum(chunks) < SHARD:
        chunks.append(min(main_chunk, SHARD - sum(chunks)))
    starts = [sum(chunks[:i]) for i in range(len(chunks))]

    PAIRS = NB // 2

    with tile.TileContext(nc) as tc:
        with (
            tc.tile_pool(name="consts", bufs=1) as cpool,
            tc.tile_pool(name="uchunks", bufs=chunk_bufs) as upool,
            tc.tile_pool(name="work", bufs=wbufs) as wpool,
            tc.tile_pool(name="outs", bufs=wbufs) as opool,
            tc.tile_pool(name="psum", bufs=2, space="PSUM") as ppool,
        ):
            # mt as one DMA: [P, KT*N] rows are 4KB contiguous -> big packets
            mt = cpool.tile([P, KT, N], fp8, tag="mt")
            nc.sync.dma_start(mt[:], mat[:])
            # u chunk tiles from a small pool: chunk c+chunk_bufs's DMA
            # waits for chunk c's matmuls (natural backpressure keeps
            # early chunks from sharing DMA bandwidth with late ones)
            chunk_map = {}  # b-tile index -> (tile, local col offset)
            pending = list(zip(starts, chunks))

            def prefetch(upto_tile):
                # emit chunk DMAs for chunks whose first b-tile <= upto_tile;
                # warmup chunks come from consts pool (no reuse), main chunks
                # from upool (bufs=chunk_bufs gives DMA backpressure)
                for st, cw in pending[:]:
                    if st // P > upto_tile:
                        break
                    wi = starts.index(st)
                    pool = cpool if wi < len(warm) else upool
                    t = pool.tile([P, KT, cw], fp8,
                                  tag=("uw%d" % wi if wi < len(warm) else "uc"),
                                  name=f"uc{st}")
                    nc.sync.dma_start(t[:], uT[:, :, st:st + cw])
                    for bb in range(st // P, (st + cw) // P):
                        chunk_map[bb] = (t, bb * P - st)
                    pending.remove((st, cw))

            PF = 8  # prefetch distance in b-tiles

            for it in range(reps):
                for i in range(PAIRS):
                    prefetch(2 * i + 1 + PF)
                    ps = ppool.tile([P, 2 * N], f32, tag="ps", name="ps")
                    for t in range(2):
                        b = 2 * i + t
                        ut, boff = chunk_map[b]
                        for ks in range(0, KT, 2):
                            for h in range(2):
                                nc.tensor.matmul(
                                    ps[:, t * N + h * 512:
                                       t * N + (h + 1) * 512],
                                    ut[:, ks:ks + 2, boff:boff + P],
                                    mt[:, ks:ks + 2, h * 512:(h + 1) * 512],
                                    start=(ks == 0),
                                    stop=(ks == KT - 2),
                                    perf_mode=DR,
                                    skip_group_check=True,
                                )
                    t16 = wpool.tile([P, 2, N], i16, tag="t16")
                    # Bresenham split of W1 between ACT and DVE; raw sums
                    # go straight out (host does & 1)
                    on_act = (i * act_pairs) % PAIRS < act_pairs
                    if on_act:
                        nc.scalar.activation(
                            t16[:], ps[:],
                            mybir.ActivationFunctionType.Copy)
                    else:
                        nc.vector.tensor_copy(t16[:], ps[:])
                    for t in range(2):
                        b = 2 * i + t
                        nc.sync.dma_start(y[b * P:(b + 1) * P, :], t16[:, t])
    nc.compile()
    return nc


def chunk_schedule(warm, main_chunk):
    chunks = list(warm)
    while sum(chunks) < SHARD:
        chunks.append(min(main_chunk, SHARD - sum(chunks)))
    starts = [sum(chunks[:i]) for i in range(len(chunks))]
    return starts, chunks


def _build_nc_v3(reps=1, warm=(256, 256, 512), main_chunk=1024,
                 chunk_bufs=3, wbufs=3, warmup_mms=40, pf=8, out_u8=True,
                 pair_dma=False, chunk_major=False, warmup_fd=512,
                 split_mt=False, alt_out_queue=False, fast_tail=0,
                 warmup_noinit=False):
    """v3: pair PSUM ([128,2048] f32, bufs=2) with W1 split across BOTH
    engines per pair (ACT evicts tile A's 1024 cols, DVE tile B's) so the
    pair frees in ~1.4us < the 2.1us matmul fill time -> PE never stalls.
    Raw i16 sums out (host does &1). Dummy warmup matmuls during the input
    lead-in keep the PE's HAM clock at 2.4GHz for the first real tiles.
    """
    import concourse.tile as tile
    from concourse import bacc, mybir

    nc = bacc.Bacc("TRN2", target_bir_lowering=False, debug=False)
    fp8 = mybir.dt.float8e4
    f32 = mybir.dt.float32
    i16 = mybir.dt.int16
    DR = mybir.MatmulPerfMode.DoubleRow

    u8 = mybir.dt.uint8
    out_dt = u8 if out_u8 else i16

    # chunk_major: host lays u out chunk-contiguous ([P, KT*cw] per chunk,
    # concatenated) so each chunk DMA is one contiguous run per partition
    uT = nc.declare_dram_parameter(
        "uT", [P, KT * SHARD] if chunk_major else [P, KT, SHARD], fp8,
        isOutput=False)
    mat = nc.declare_dram_parameter("mat", [P, KT, N], fp8, isOutput=False)
    # raw sums out: u8 saturating (host certifies no 255 appeared -> exact,
    # else reruns the i16 build) or i16 exact
    y = nc.declare_dram_parameter("y", [SHARD, N], out_dt, isOutput=True)

    starts, chunks = chunk_schedule(warm, main_chunk)
    PAIRS = NB // 2

    with tile.TileContext(nc) as tc:
        with (
            tc.tile_pool(name="consts", bufs=1) as cpool,
            tc.tile_pool(name="uchunks", bufs=chunk_bufs) as upool,
            tc.tile_pool(name="work", bufs=wbufs) as wpool,
            tc.tile_pool(name="psum", bufs=4, space="PSUM") as ppool,
        ):
            # PE warmup: dummy matmuls with no DMA deps keep the HAM busy
            # window hot while inputs stream in. Scratch operands from a
            # memset tile (DVE memsets it right after the preamble); results
            # land in a psum buf that a later tile overwrites (start=True).
            # warmup_fd tunes per-MM duration so the warmup block ends just
            # as the first input chunk lands (queue order gates real MMs).
            if warmup_mms:
                wfd = max(warmup_fd, P)
                scratch = cpool.tile([P, 2, wfd], fp8, tag="scratch")
                if not warmup_noinit:
                    nc.vector.memset(scratch[:], 0)
                # warmup_noinit: read uninitialized SBUF (garbage values are
                # fine -- warmup psum results are discarded and overwritten
                # with start=True) so the PE starts ~2us earlier, right
                # after its own preamble instead of after DVE's memset
                wp = ppool.tile([P, N], f32, tag="ps", name="ps_warm")
                for _ in range(warmup_mms):
                    nc.tensor.matmul(wp[:, 0:wfd], scratch[:, :, 0:P],
                                     scratch[:], start=True, stop=True,
                                     perf_mode=DR, skip_group_check=True)

            # mt split by ks-pairs: the first tiles' start-group matmuls only
            # need ks 0-1 (256KB), so they launch ~1.3us before the full
            # 512KB would have landed; ks 2-3 arrives while they run
            chunk_map = {}
            pending = list(zip(starts, chunks))

            if split_mt:
                mt01 = cpool.tile([P, 2, N], fp8, tag="mt01")
                mt23 = cpool.tile([P, 2, N], fp8, tag="mt23")
                if split_mt == "fine":
                    # two parallel DMAs for mt01 double its share of the
                    # round-robin DMA ring bandwidth -> first matmul earlier
                    nc.sync.dma_start(mt01[:, 0:1, :], mat[:, 0:1, :])
                    nc.sync.dma_start(mt01[:, 1:2, :], mat[:, 1:2, :])
                elif split_mt == "h":
                    # h-halves: tile 0's first matmul reads only cols 0-511
                    # of mt01 (region-tracked), gating on 128KB not 256KB
                    nc.sync.dma_start(mt01[:, :, 0:512], mat[:, 0:2, 0:512])
                else:
                    nc.sync.dma_start(mt01[:], mat[:, 0:2, :])
                mt_of = {0: (mt01, 0), 2: (mt23, 0)}
            else:
                mt = cpool.tile([P, KT, N], fp8, tag="mt")
                nc.sync.dma_start(mt[:], mat[:])
                mt_of = {0: (mt, 0), 2: (mt, 2)}

            def prefetch(upto_tile):
                for st, cw in pending[:]:
                    if st // P > upto_tile:
                        break
                    wi = starts.index(st)
                    pool = cpool if wi < len(warm) else upool
                    t = pool.tile([P, KT, cw], fp8,
                                  tag=("uw%d" % wi if wi < len(warm) else "uc"),
                                  name=f"uc{st}")
                    if chunk_major:
                        off = KT * st
                        src = uT[:, off:off + KT * cw].rearrange(
                            "p (k c) -> p k c", k=KT)
                    else:
                        src = uT[:, :, st:st + cw]
                    nc.sync.dma_start(t[:], src)
                    for bb in range(st // P, (st + cw) // P):
                        chunk_map[bb] = (t, bb * P - st)
                    pending.remove((st, cw))

            if split_mt:
                prefetch(0)  # chunk0 lands right behind mt01
                if split_mt == "h":
                    nc.sync.dma_start(mt01[:, :, 512:N], mat[:, 0:2, 512:N])
                nc.sync.dma_start(mt23[:], mat[:, 2:4, :])

            for it in range(reps):
                for b in range(NB):
                    prefetch(b + pf)
                    ps = ppool.tile([P, N], f32, tag="ps", name="ps")
                    ut, boff = chunk_map[b]
                    for ks in range(0, KT, 2):
                        mtt, mks = mt_of[ks]
                        for h in range(2):
                            nc.tensor.matmul(
                                ps[:, h * 512:(h + 1) * 512],
                                ut[:, ks:ks + 2, boff:boff + P],
                                mtt[:, mks:mks + 2, h * 512:(h + 1) * 512],
                                start=(ks == 0),
                                stop=(ks == KT - 2),
                                perf_mode=DR,
                                skip_group_check=True,
                            )
                    # W1 alternates engines per tile; 4-deep psum pipeline
                    # absorbs eviction latency jitter
                    if pair_dma:
                        if b % 2 == 0:
                            t16p = wpool.tile([P, 2, N], out_dt, tag="t16")
                            nc.scalar.activation(
                                t16p[:, 0], ps[:],
                                mybir.ActivationFunctionType.Copy)
                        else:
                            nc.vector.tensor_copy(t16p[:, 1], ps[:])
                            dst = y[(b - 1) * P:(b + 1) * P, :].rearrange(
                                "(t p) n -> p t n", t=2)
                            nc.sync.dma_start(dst, t16p[:])
                        continue
                    t16 = wpool.tile([P, N], out_dt, tag="t16")
                    if b >= NB - fast_tail:
                        # tail tiles: split the evict across BOTH engines
                        # (different psum banks) + 2 half-DMAs so the final
                        # serial chain after the last matmul is shorter
                        nc.scalar.activation(t16[:, 0:512], ps[:, 0:512],
                                             mybir.ActivationFunctionType.Copy)
                        nc.vector.tensor_copy(t16[:, 512:N], ps[:, 512:N])
                        nc.sync.dma_start(y[b * P:(b + 1) * P, 0:512],
                                          t16[:, 0:512])
                        nc.sync.dma_start(y[b * P:(b + 1) * P, 512:N],
                                          t16[:, 512:N])
                        continue
                    if b % 2 == 0:
                        nc.scalar.activation(t16[:], ps[:],
                                             mybir.ActivationFunctionType.Copy)
                    else:
                        nc.vector.tensor_copy(t16[:], ps[:])
                    # odd tiles' out-DMA issues from the ACT queue (HWDGE on
                    # either SP or ACT) -> halves SP descriptor pacing
                    eng = nc.scalar if (alt_out_queue and b % 2 == 1) else nc.sync
                    eng.dma_start(y[b * P:(b + 1) * P, :], t16[:])
    nc.compile()
    return nc


def _build_nc_v4(reps=1, warm=(128, 256, 512), main_chunk=2048,
                 chunk_bufs=3, wbufs=8, warmup_mms=5, pf=12, out_u8=True,
                 warmup_fd=512, warmup_noinit=False, fast_tail=2,
                 chunk_major=True, mt_q="scalar", chunk_q="gpsimd",
                 out_q=("sync", "gpsimd"), split_mt="h", psum_bufs=4,
                 dummy_dma=False, evict_split="alt"):
    """v4/v5 experiments on top of v3.

    Measured v4 lesson (multi-queue: mt on ACT, chunks on Pool, outs on
    SP+Pool): DMA engines round-robin across ALL queues with pending
    descriptors, so extra queues give no prioritization (v3's single-queue
    FIFO order IS the priority mechanism), and every extra HWDGE queue
    adds ~1.1us to the fixed NEFF teardown (postamble queue reset). ->
    v5 reverts to a single SP queue for everything.

    v5 additions:
      - dummy_dma: a 1-byte DMA as the first SP op wakes the DMA engine
        rings (~0.8us spin-up) during the descriptor gen of the real
        first loads.
      - chunk_major: u laid out chunk-contiguous so warm-chunk DMA
        packets are >=512B (the [P,KT,cw] layout gives cw-byte packets:
        128B for the first warm chunk, ~5GB/s/engine).
      - evict_split="tile": EVERY tile's eviction splits into ACT half +
        DVE half (~630ns each) instead of alternating whole-tile
        evictions (1114/1224ns): mid-phase traces show PSUM-release
        backpressure stalls (MM waits on eviction sems) with the
        alternating scheme.
    """
    import concourse.tile as tile
    from concourse import bacc, mybir

    nc = bacc.Bacc("TRN2", target_bir_lowering=False, debug=False)
    fp8 = mybir.dt.float8e4
    f32 = mybir.dt.float32
    i16 = mybir.dt.int16
    DR = mybir.MatmulPerfMode.DoubleRow

    u8 = mybir.dt.uint8
    out_dt = u8 if out_u8 else i16

    uT = nc.declare_dram_parameter(
        "uT", [P, KT * SHARD] if chunk_major else [P, KT, SHARD], fp8,
        isOutput=False)
    mat = nc.declare_dram_parameter("mat", [P, KT, N], fp8, isOutput=False)
    y = nc.declare_dram_parameter("y", [SHARD, N], out_dt, isOutput=True)

    starts, chunks = chunk_schedule(warm, main_chunk)

    with tile.TileContext(nc) as tc:
        eng = {"sync": nc.sync, "scalar": nc.scalar, "vector": nc.vector,
               "gpsimd": nc.gpsimd, "tensor": nc.tensor}
        mtq = eng[mt_q]
        ckq = eng[chunk_q]
        oq0, oq1 = eng[out_q[0]], eng[out_q[1]]
        with (
            tc.tile_pool(name="consts", bufs=1) as cpool,
            tc.tile_pool(name="uchunks", bufs=chunk_bufs) as upool,
            tc.tile_pool(name="work", bufs=wbufs) as wpool,
            tc.tile_pool(name="psum", bufs=psum_bufs, space="PSUM") as ppool,
        ):
            if dummy_dma:
                # 1-byte DMA as the first queue op: rings spin up (~0.8us)
                # while the real loads' descriptors generate
                wake = cpool.tile([1, 1], fp8, tag="wake")
                mtq.dma_start(wake[:], mat[0:1, 0:1, 0:1])
            # mt h-split so tile0's first matmul gates on 128KB
            # (region-level tracking), rest streams behind
            mt01 = cpool.tile([P, 2, N], fp8, tag="mt01")
            mt23 = cpool.tile([P, 2, N], fp8, tag="mt23")
            if split_mt == "h":
                mtq.dma_start(mt01[:, :, 0:512], mat[:, 0:2, 0:512])
            else:
                mtq.dma_start(mt01[:], mat[:, 0:2, :])
            mt_of = {0: (mt01, 0), 2: (mt23, 0)}

            chunk_map = {}
            pending = list(zip(starts, chunks))

            def prefetch(upto_tile):
                for st, cw in pending[:]:
                    if st // P > upto_tile:
                        break
                    wi = starts.index(st)
                    pool = cpool if wi < len(warm) else upool
                    t = pool.tile([P, KT, cw], fp8,
                                  tag=("uw%d" % wi if wi < len(warm) else "uc"),
                                  name=f"uc{st}")
                    if chunk_major:
                        off = KT * st
                        src = uT[:, off:off + KT * cw].rearrange(
                            "p (k c) -> p k c", k=KT)
                    else:
                        src = uT[:, :, st:st + cw]
                    ckq.dma_start(t[:], src)
                    for bb in range(st // P, (st + cw) // P):
                        chunk_map[bb] = (t, bb * P - st)
                    pending.remove((st, cw))

            prefetch(0)  # chunk0 on its own queue, parallel with mt01
            if split_mt == "h":
                mtq.dma_start(mt01[:, :, 512:N], mat[:, 0:2, 512:N])
            mtq.dma_start(mt23[:], mat[:, 2:4, :])

            # PE warmup: dummy matmuls (no DMA deps) hold the HAM clock
            # hot while the first inputs stream in
            if warmup_mms:
                wfd = max(warmup_fd, P)
                scratch = cpool.tile([P, 2, wfd], fp8, tag="scratch")
                if not warmup_noinit:
                    nc.vector.memset(scratch[:], 0)
                wp = ppool.tile([P, N], f32, tag="ps", name="ps_warm")
                for _ in range(warmup_mms):
                    nc.tensor.matmul(wp[:, 0:wfd], scratch[:, :, 0:P],
                                     scratch[:], start=True, stop=True,
                                     perf_mode=DR, skip_group_check=True)

            for it in range(reps):
                for b in range(NB):
                    prefetch(b + pf)
                    ps = ppool.tile([P, N], f32, tag="ps", name="ps")
                    ut, boff = chunk_map[b]
                    for ks in range(0, KT, 2):
                        mtt, mks = mt_of[ks]
                        for h in range(2):
                            nc.tensor.matmul(
                                ps[:, h * 512:(h + 1) * 512],
                                ut[:, ks:ks + 2, boff:boff + P],
                                mtt[:, mks:mks + 2, h * 512:(h + 1) * 512],
                                start=(ks == 0),
                                stop=(ks == KT - 2),
                                perf_mode=DR,
                                skip_group_check=True,
                            )
                    t16 = wpool.tile([P, N], out_dt, tag="t16")
                    if evict_split == "tile" or b >= NB - fast_tail:
                        # eviction split across BOTH engines (different
                        # psum banks), one output DMA waiting on both
                        nc.scalar.activation(t16[:, 0:512], ps[:, 0:512],
                                             mybir.ActivationFunctionType.Copy)
                        nc.vector.tensor_copy(t16[:, 512:N], ps[:, 512:N])
                        oq = oq0 if b % 2 == 0 else oq1
                        oq.dma_start(y[b * P:(b + 1) * P, :], t16[:])
                        continue
                    if b % 2 == 0:
                        nc.scalar.activation(t16[:], ps[:],
                                             mybir.ActivationFunctionType.Copy)
                        oq0.dma_start(y[b * P:(b + 1) * P, :], t16[:])
                    else:
                        nc.vector.tensor_copy(t16[:], ps[:])
                        oq1.dma_start(y[b * P:(b + 1) * P, :], t16[:])
    nc.compile()
    return nc


def _build_nc_v6(reps=1, warm=(128, 256, 512), main_chunk=2048,
                 chunk_bufs=3, wbufs=4, warmup_mms=5, pf=12, out_u8=True,
                 warmup_fd=512, group=4, in_pkt=4096, dummy_dma=True,
                 tail_splits=(2048, 1024, 1024), psum_bufs=4,
                 warmup_noinit=False, pf_bottom=False,
                 gate_min_wi=99, gate_lead=7, chunk_lead=None):
    """v6: output DMA packet-size fix.

    Trace evidence: output DMAs ([128,1024] u8 -> 1KB DRAM rows) sustain
    only ~180GB/s (per-packet overhead ~45ns + 46ns transfer per 1KB), so
    the output stream (needs 148GB/s avg, bursts when inputs compete)
    backlogs and the drain runs ~8us past the last matmul. Fix: group
    `group` consecutive b-tiles into one DMA with DRAM layout
    [NB/group, P, group*1024] (partition p's rows from `group` tiles
    contiguous -> group-KB packets, ~300GB/s at 4KB). Host reassembles
    with a transpose (it already does &1). Input chunk packets capped at
    `in_pkt` bytes via AP grouping so round-robin stays ~fair.

    Tail taper: the last group's DMA is split by `tail_splits` (bytes of
    the group's 4096-col span per sub-DMA, last entries = the last
    tiles) so the final serial chain after the last matmul is short; the
    last two tiles' evictions split across ACT+DVE halves.
    """
    import concourse.tile as tile
    from concourse import bacc, mybir
    from concourse.tile_rust import add_dep_helper

    nc = bacc.Bacc("TRN2", target_bir_lowering=False, debug=False)
    fp8 = mybir.dt.float8e4
    f32 = mybir.dt.float32
    i16 = mybir.dt.int16
    DR = mybir.MatmulPerfMode.DoubleRow

    u8 = mybir.dt.uint8
    out_dt = u8 if out_u8 else i16
    GN = group * N          # output columns per group row
    NG = NB // group        # number of groups

    # chunk-major u layout (contiguous per chunk)
    uT = nc.declare_dram_parameter("uT", [P, KT * SHARD], fp8, isOutput=False)
    mat = nc.declare_dram_parameter("mat", [P, KT, N], fp8, isOutput=False)
    y = nc.declare_dram_parameter("y", [NG, P, GN], out_dt, isOutput=True)

    starts, chunks = chunk_schedule(warm, main_chunk)

    with tile.TileContext(nc) as tc:
        with (
            tc.tile_pool(name="consts", bufs=1) as cpool,
            tc.tile_pool(name="uchunks", bufs=chunk_bufs) as upool,
            tc.tile_pool(name="work", bufs=wbufs) as wpool,
            tc.tile_pool(name="psum", bufs=psum_bufs, space="PSUM") as ppool,
        ):
            if dummy_dma:
                wake = cpool.tile([1, 1], fp8, tag="wake")
                nc.sync.dma_start(wake[:], mat[0:1, 0:1, 0:1])
            mt01 = cpool.tile([P, 2, N], fp8, tag="mt01")
            mt23 = cpool.tile([P, 2, N], fp8, tag="mt23")
            nc.sync.dma_start(mt01[:, :, 0:512], mat[:, 0:2, 0:512])
            mt_of = {0: (mt01, 0), 2: (mt23, 0)}

            chunk_map = {}
            pending = list(zip(starts, chunks))
            last_mm = {}   # tile index -> last matmul instruction of tile
            warm_gate = [None]  # last warmup matmul

            def prefetch(upto_tile):
                for st, cw in pending[:]:
                    if st // P > upto_tile:
                        break
                    wi = starts.index(st)
                    pool = cpool if wi < len(warm) else upool
                    t = pool.tile([P, KT, cw], fp8,
                                  tag=("uw%d" % wi if wi < len(warm) else "uc"),
                                  name=f"uc{st}")
                    off = KT * st
                    src = uT[:, off:off + KT * cw].rearrange(
                        "p (k c) -> p k c", k=KT)
                    dma = nc.sync.dma_start(t[:], src)
                    # hold big chunks off the wire until the PE reaches a
                    # matmul ~gate_lead tiles before the chunk is needed:
                    # without this they hit the DMA engines immediately
                    # (8KB packets out-compete the 2KB mt transfers the
                    # first tiles gate on in the per-packet round-robin)
                    if wi >= gate_min_wi:
                        gt = st // P - gate_lead
                        gate = None
                        emitted = [bb for bb in last_mm if bb <= gt]
                        if emitted:
                            gate = last_mm[max(emitted)]
                        elif last_mm:
                            gate = last_mm[min(last_mm)]
                        else:
                            gate = warm_gate[0]
                        if gate is not None:
                            add_dep_helper(
                                dma.ins, gate.ins,
                                reason="hold chunk DMA until PE progress")
                    for bb in range(st // P, (st + cw) // P):
                        chunk_map[bb] = (t, bb * P - st)
                    pending.remove((st, cw))

            prefetch(0)
            nc.sync.dma_start(mt01[:, :, 512:N], mat[:, 0:2, 512:N])
            nc.sync.dma_start(mt23[:], mat[:, 2:4, :])
            if chunk_lead is not None:
                # emit the remaining warm chunks (ungated — needed at
                # tiles 1..warm_end and small enough not to hog the wire)
                prefetch(sum(warm) // P - 1)

            if warmup_mms:
                wfd = max(warmup_fd, P)
                scratch = cpool.tile([P, 2, wfd], fp8, tag="scratch")
                if not warmup_noinit:
                    nc.vector.memset(scratch[:], 0)
                wp = ppool.tile([P, N], f32, tag="ps", name="ps_warm")
                for _ in range(warmup_mms):
                    warm_gate[0] = nc.tensor.matmul(
                        wp[:, 0:wfd], scratch[:, :, 0:P],
                        scratch[:], start=True, stop=True,
                        perf_mode=DR, skip_group_check=True)

            # chunk_lead mode: main chunks (wi >= len(warm)) are emitted
            # right after the out-DMA of group (start_tile-chunk_lead)//
            # group, whose eviction wait blocks the SP queue head — this
            # holds the 8KB-packet chunk transfers off the wire (they
            # out-compete mt/warm loads in per-packet round-robin)
            # without any extra instructions or dependency surgery.
            chunk_after_group = {}
            if chunk_lead is not None:
                for st, cw in list(pending):
                    wi = starts.index(st)
                    if wi < len(warm):
                        continue
                    gk = max(0, (st // P - chunk_lead)) // group
                    chunk_after_group.setdefault(gk, []).append(st // P)

            for it in range(reps):
                tq = None
                for b in range(NB):
                    if not pf_bottom and chunk_lead is None:
                        prefetch(b + pf)
                    ps = ppool.tile([P, N], f32, tag="ps", name="ps")
                    ut, boff = chunk_map[b]
                    for ks in range(0, KT, 2):
                        mtt, mks = mt_of[ks]
                        for h in range(2):
                            last_mm[b] = nc.tensor.matmul(
                                ps[:, h * 512:(h + 1) * 512],
                                ut[:, ks:ks + 2, boff:boff + P],
                                mtt[:, mks:mks + 2, h * 512:(h + 1) * 512],
                                start=(ks == 0),
                                stop=(ks == KT - 2),
                                perf_mode=DR,
                                skip_group_check=True,
                            )
                    g, t = divmod(b, group)
                    if t == 0:
                        tq = wpool.tile([P, GN], out_dt, tag="tq")
                    dst_col = t * N
                    last_group = g == NG - 1
                    if last_group and t >= group - 2:
                        # final two tiles: halves on both engines
                        nc.scalar.activation(
                            tq[:, dst_col:dst_col + 512], ps[:, 0:512],
                            mybir.ActivationFunctionType.Copy)
                        nc.vector.tensor_copy(
                            tq[:, dst_col + 512:dst_col + N], ps[:, 512:N])
                    elif b % 2 == 0:
                        nc.scalar.activation(
                            tq[:, dst_col:dst_col + N], ps[:],
                            mybir.ActivationFunctionType.Copy)
                    else:
                        nc.vector.tensor_copy(
                            tq[:, dst_col:dst_col + N], ps[:])
                    if last_group and tail_splits:
                        # emit each sub-DMA right after the tile that
                        # completes its span, so issue overlaps the
                        # remaining matmuls and the final chain is short
                        ends, acc = [], GN - sum(tail_splits)
                        for w in tail_splits:
                            acc += w
                            ends.append(acc)
                        done_col = (t + 1) * N
                        col0 = GN - sum(tail_splits)
                        if t == 0 and col0:
                            pass  # head span handled when its end tile evicts
                        for i_s, e in enumerate(ends):
                            if e == done_col:
                                s = (ends[i_s - 1] if i_s else col0)
                                if i_s == 0 and col0:
                                    s = 0  # fold the head span into split 0
                                nc.sync.dma_start(y[g, :, s:e],
                                                  tq[:, s:e])
                    elif t == group - 1:
                        nc.sync.dma_start(y[g, :, :], tq[:])
                    if t == group - 1 and chunk_lead is not None:
                        for st_tile in chunk_after_group.get(g, []):
                            prefetch(st_tile)
                    if pf_bottom:
                        # emit chunk DMAs AFTER this tile's output DMA:
                        # the out-DMA's eviction-wait blocks the SP queue
                        # head, so a main chunk can't hit the wire early
                        # and steal engine bandwidth from mt/warm-chunk
                        # loads (queue FIFO orders starts, transfers
                        # overlap otherwise)
                        prefetch(b + 1 + pf)
    nc.compile()
    return nc


# v7 input blob layout (bytes per partition, in stream order):
# [c0 512 | mt01 2048 | mt23 2048 | c1 1024 | c2 2048 | c3 8192 |
#  c4 8192 | c5 8192 | c6 4608]  -> total 36864 = KT*SHARD + KT*N
V7_WARM = (128, 256, 512)           # c0..c2 batch widths
V7_MAIN = (2048, 2048, 2048, 1152)  # c3..c6
V7_OFF = {}
_o = 0
for _nm, _w in [("c0", 512), ("mt01", 2048), ("mt23", 2048),
                ("c1", 1024), ("c2", 2048), ("c3", 8192),
                ("c4", 8192), ("c5", 8192), ("c6", 4608)]:
    V7_OFF[_nm] = (_o, _o + _w)
    _o += _w
V7_TOTAL = _o


def _build_nc_v7(reps=1, wbufs=4, warmup_mms=20, warmup_fd=128, out_u8=True,
                 group=4, tail_splits=(2048, 1024, 1024), psum_bufs=4,
                 dummy_dma=True, chunk_bufs=2, out_q="sync",
                 chunk_after=(0, 1, 2), psum_half=False,
                 front_split=False):
    """v7: single-FIFO-queue schedule built from measured DMA behavior.

    Measured: DMAs on one queue transfer strictly FIFO (no bandwidth
    stealing), but each DMA instruction costs ~0.5-0.6us of dead handoff
    before its packets flow. So the early loads are packed into a host-
    side contiguous blob and fetched as 3 big DMAs ([c0|mt01], [mt23],
    [c1|c2]) instead of 6 small ones, and the main chunks are emitted
    between output-group DMAs so the FIFO position (not semaphores)
    paces them. Outputs grouped `group` tiles per DMA (4KB packets,
    ~300GB/s vs ~180 at 1KB). PE warmup uses a small scratch (short
    memset -> warmups from ~7.4us) so the clock ramp finishes early.
    """
    import concourse.tile as tile
    from concourse import bacc, mybir

    nc = bacc.Bacc("TRN2", target_bir_lowering=False, debug=False)
    fp8 = mybir.dt.float8e4
    f32 = mybir.dt.float32
    i16 = mybir.dt.int16
    DR = mybir.MatmulPerfMode.DoubleRow

    u8 = mybir.dt.uint8
    out_dt = u8 if out_u8 else i16
    GN = group * N
    NG = NB // group

    blob = nc.declare_dram_parameter("blob", [P, V7_TOTAL], fp8,
                                     isOutput=False)
    y = nc.declare_dram_parameter("y", [NG, P, GN], out_dt, isOutput=True)

    warm_starts = [0, 128, 384]
    main_starts = [896, 2944, 4992, 7040]

    with tile.TileContext(nc) as tc:
        oq = {"sync": nc.sync, "scalar": nc.scalar,
              "gpsimd": nc.gpsimd}[out_q]
        with (
            tc.tile_pool(name="consts", bufs=1) as cpool,
            tc.tile_pool(name="uchunks", bufs=chunk_bufs) as upool,
            tc.tile_pool(name="work", bufs=wbufs) as wpool,
            tc.tile_pool(name="psum", bufs=psum_bufs, space="PSUM") as ppool,
        ):
            if dummy_dma:
                wake = cpool.tile([1, 1], fp8, tag="wake")
                nc.sync.dma_start(wake[:], blob[0:1, 0:1])
            if front_split:
                # [c0|mt01] then [mt23|c1]: consecutive DMAs overlap on
                # the engines (measured), so total wire time is the same
                # but tile 0's ks01 matmuls start ~1.5us earlier, during
                # the clock ramp, and c1 rides mt23's semaphore.
                big1 = cpool.tile([P, 2560], fp8, tag="big1")
                big2 = cpool.tile([P, 3072], fp8, tag="big2")
                w2 = cpool.tile([P, 2048], fp8, tag="w2")
                nc.sync.dma_start(big1[:], blob[:, 0:2560])
                nc.sync.dma_start(big2[:], blob[:, 2560:5632])
                nc.sync.dma_start(w2[:], blob[:, 5632:7680])
                mt01v = big1[:, 512:2560].rearrange("p (k n) -> p k n", k=2)
                mt23v = big2[:, 0:2048].rearrange("p (k n) -> p k n", k=2)
                mt_of = {0: mt01v, 2: mt23v}
                chunk_map = {}
                c0v = big1[:, 0:512].rearrange("p (k c) -> p k c", k=KT)
                chunk_map[0] = (c0v, 0)
                c1v = big2[:, 2048:3072].rearrange("p (k c) -> p k c", k=KT)
                for b in (1, 2):
                    chunk_map[b] = (c1v, b * P - 128)
                c2v = w2[:].rearrange("p (k c) -> p k c", k=KT)
                for b in range(3, 7):
                    chunk_map[b] = (c2v, b * P - 384)
            else:
                big1 = cpool.tile([P, 4608], fp8, tag="big1")
                w12 = cpool.tile([P, 3072], fp8, tag="w12")
                nc.sync.dma_start(big1[:], blob[:, 0:4608])
                nc.sync.dma_start(w12[:, 0:1024], blob[:, 4608:5632])
                nc.sync.dma_start(w12[:, 1024:3072], blob[:, 5632:7680])
                mt01v = big1[:, 512:2560].rearrange("p (k n) -> p k n", k=2)
                mt23v = big1[:, 2560:4608].rearrange("p (k n) -> p k n", k=2)
                mt_of = {0: mt01v, 2: mt23v}
                chunk_map = {}
                c0v = big1[:, 0:512].rearrange("p (k c) -> p k c", k=KT)
                chunk_map[0] = (c0v, 0)
                c1v = w12[:, 0:1024].rearrange("p (k c) -> p k c", k=KT)
                for b in (1, 2):
                    chunk_map[b] = (c1v, b * P - 128)
                c2v = w12[:, 1024:3072].rearrange("p (k c) -> p k c", k=KT)
                for b in range(3, 7):
                    chunk_map[b] = (c2v, b * P - 384)

            main_pending = []
            for ci, st in enumerate(main_starts):
                nm = f"c{ci + 3}"
                o0, o1 = V7_OFF[nm]
                cw = (o1 - o0) // KT
                main_pending.append((st, cw, o0, o1, nm))

            def emit_main():
                st, cw, o0, o1, nm = main_pending.pop(0)
                t = upool.tile([P, KT, cw], fp8, tag="uc", name=nm)
                nc.sync.dma_start(
                    t[:], blob[:, o0:o1].rearrange("p (k c) -> p k c", k=KT))
                for bb in range(st // P, st // P + cw // P):
                    chunk_map[bb] = (t, bb * P - st)

            emit_main()  # c3 right behind the warm loads in FIFO

            if warmup_mms:
                wfd = max(warmup_fd, P)
                scratch = cpool.tile([P, 2, wfd], fp8, tag="scratch")
                nc.vector.memset(scratch[:], 0)
                wp = ppool.tile([P, 512 if psum_half else N], f32,
                                tag="ps", name="ps_warm")
                for _ in range(warmup_mms):
                    nc.tensor.matmul(wp[:, 0:wfd], scratch[:, :, 0:P],
                                     scratch[:, :, 0:wfd], start=True,
                                     stop=True,
                                     perf_mode=DR, skip_group_check=True)

            for it in range(reps):
                tq = None
                for b in range(NB):
                    g, t = divmod(b, group)
                    if t == 0:
                        tq = wpool.tile([P, GN], out_dt, tag="tq")
                    dst_col = t * N
                    ut, boff = chunk_map[b]
                    if psum_half:
                        # per-half psum tiles (1 bank, 8 bufs): finer
                        # release granularity; h0 evicts on ACT, h1 on
                        # DVE every tile (~620ns each per 864ns cadence).
                        # ks-outer order: mt23 is only needed at MM3, so
                        # tile 0 starts as soon as [c0|mt01] lands.
                        pshs = [ppool.tile([P, 512], f32, tag="ps",
                                           name="ps") for _ in range(2)]
                        for ks in range(0, KT, 2):
                            for h in range(2):
                                nc.tensor.matmul(
                                    pshs[h][:],
                                    ut[:, ks:ks + 2, boff:boff + P],
                                    mt_of[ks][:, 0:2,
                                              h * 512:(h + 1) * 512],
                                    start=(ks == 0),
                                    stop=(ks == KT - 2),
                                    perf_mode=DR,
                                    skip_group_check=True,
                                )
                        nc.scalar.activation(
                            tq[:, dst_col:dst_col + 512], pshs[0][:],
                            mybir.ActivationFunctionType.Copy)
                        nc.vector.tensor_copy(
                            tq[:, dst_col + 512:dst_col + N], pshs[1][:])
                        last_group = g == NG - 1
                    else:
                        ps = ppool.tile([P, N], f32, tag="ps", name="ps")
                        for ks in range(0, KT, 2):
                            mtt = mt_of[ks]
                            for h in range(2):
                                nc.tensor.matmul(
                                    ps[:, h * 512:(h + 1) * 512],
                                    ut[:, ks:ks + 2, boff:boff + P],
                                    mtt[:, 0:2, h * 512:(h + 1) * 512],
                                    start=(ks == 0),
                                    stop=(ks == KT - 2),
                                    perf_mode=DR,
                                    skip_group_check=True,
                                )
                        last_group = g == NG - 1
                        if last_group and t >= group - 2:
                            nc.scalar.activation(
                                tq[:, dst_col:dst_col + 512], ps[:, 0:512],
                                mybir.ActivationFunctionType.Copy)
                            nc.vector.tensor_copy(
                                tq[:, dst_col + 512:dst_col + N],
                                ps[:, 512:N])
                        elif b % 2 == 0:
                            nc.scalar.activation(
                                tq[:, dst_col:dst_col + N], ps[:],
                                mybir.ActivationFunctionType.Copy)
                        else:
                            nc.vector.tensor_copy(
                                tq[:, dst_col:dst_col + N], ps[:])
                    if last_group and tail_splits:
                        ends, acc = [], GN - sum(tail_splits)
                        for w in tail_splits:
                            acc += w
                            ends.append(acc)
                        done_col = (t + 1) * N
                        col0 = GN - sum(tail_splits)
                        for i_s, e in enumerate(ends):
                            if done_col - N < e <= done_col:
                                s = (ends[i_s - 1] if i_s else col0)
                                if i_s == 0 and col0:
                                    s = 0
                                nc.sync.dma_start(y[g, :, s:e], tq[:, s:e])
                    elif t == group - 1:
                        oq.dma_start(y[g, :, :], tq[:])
                        if g in chunk_after and main_pending:
                            emit_main()
    nc.compile()
    return nc


EVICT = "i16out"
W1_ACT = 48      # 48/64 PSUM->i16 converts on ACT, 16/64 on DVE
U_CHUNKS = 8     # input u loaded in 8 chunks so matmuls start early
WBUFS = 6
KS_OUTER = True  # k-pair outer loop: one LDWEIGHTS serves both psum halves

VERSION = 55
V7_OPTS_A = dict(wbufs=4, warmup_mms=20, warmup_fd=128, group=4,
                 tail_splits=(2048, 1024, 1024), dummy_dma=True,
                 chunk_bufs=2, out_q="sync", chunk_after=(0, 1, 2))
V7_OPTS_B = dict(wbufs=4, warmup_mms=28, warmup_fd=128, group=4,
                 tail_splits=(2048, 1024, 1024), dummy_dma=False,
                 chunk_bufs=2, out_q="sync", chunk_after=(0, 1, 2),
                 psum_half=True, psum_bufs=8)
V7_OPTS_C = dict(wbufs=4, warmup_mms=28, warmup_fd=128, group=4,
                 tail_splits=(2048, 1024, 512, 512), dummy_dma=False,
                 chunk_bufs=2, out_q="sync", chunk_after=(0, 1, 2),
                 psum_half=True, psum_bufs=8)
V7_OPTS_D = dict(wbufs=4, warmup_mms=31, warmup_fd=128, group=4,
                 tail_splits=(2048, 1024, 1024), dummy_dma=False,
                 chunk_bufs=2, out_q="sync", chunk_after=(0, 1, 2),
                 psum_half=True, psum_bufs=8)
V7_OPTS_E = dict(wbufs=4, warmup_mms=28, warmup_fd=128, group=4,
                 tail_splits=(2048, 1024, 1024), dummy_dma=True,
                 chunk_bufs=2, out_q="sync", chunk_after=(0, 1, 2),
                 psum_half=True, psum_bufs=8)
V7_OPTS_F = dict(wbufs=4, warmup_mms=20, warmup_fd=128, group=4,
                 tail_splits=(2048, 1024, 1024), dummy_dma=False,
                 chunk_bufs=2, out_q="sync", chunk_after=(0, 1, 2),
                 psum_half=True, psum_bufs=8, front_split=True)
V6_OPTS_A = dict(warm=(128, 256, 512), main_chunk=2048,
                 chunk_bufs=3, wbufs=4, warmup_mms=5, pf=12,
                 group=4, in_pkt=4096, dummy_dma=True,
                 tail_splits=(2048, 1024, 1024))
# v6b: chunk DMAs gated on PE progress (explicit deps) + uninterrupted
# warmups from ~7.2us (no memset gate) so the PE clock ramp completes
V6_OPTS_B = dict(warm=(128, 256, 512), main_chunk=2048,
                 chunk_bufs=3, wbufs=4, warmup_mms=6, pf=12,
                 group=4, in_pkt=4096, dummy_dma=True,
                 tail_splits=(2048, 1024, 1024),
                 chunk_lead=12)
V4_OPTS_A = dict(warm=(128, 256, 512), main_chunk=2048,
                 chunk_bufs=3, wbufs=8, warmup_mms=5, pf=12,
                 chunk_major=True, fast_tail=2)
# v5: single SP queue (extra HWDGE queues cost ~1.1us teardown each),
# chunk-major packets, ring-wake dummy DMA, per-tile split evictions
V5_OPTS_A = dict(warm=(128, 256, 512), main_chunk=2048,
                 chunk_bufs=3, wbufs=8, warmup_mms=6, pf=12,
                 chunk_major=True, fast_tail=0, dummy_dma=True,
                 evict_split="tile", mt_q="sync", chunk_q="sync",
                 out_q=("sync", "sync"))
V2_OPTS = dict(act_pairs=17, warm=(256, 256, 512), main_chunk=1024,
               chunk_bufs=3, wbufs=3, mt_splits=1)
V3_OPTS = dict(warm=(256, 256, 512), main_chunk=1024,
               chunk_bufs=3, wbufs=6, warmup_mms=12, pf=8)
V3_OPTS_B = dict(warm=(128, 256, 512, 1024), main_chunk=1024,
                 chunk_bufs=3, wbufs=8, warmup_mms=22, pf=8)
V3_OPTS_C = dict(warm=(128, 256, 512), main_chunk=2048,
                 chunk_bufs=3, wbufs=8, warmup_mms=10, pf=12)
V3_OPTS_D = dict(warm=(128, 256, 512), main_chunk=2048,
                 chunk_bufs=3, wbufs=8, warmup_mms=6, pf=12)
V3_OPTS_E = dict(warm=(128, 256, 512), main_chunk=2048,
                 chunk_bufs=3, wbufs=4, warmup_mms=10, pf=12, pair_dma=True)
V3_OPTS_F = dict(warm=(128, 256, 512), main_chunk=2048,
                 chunk_bufs=3, wbufs=8, warmup_mms=10, pf=12,
                 chunk_major=True)
V3_OPTS_G = dict(warm=(128, 256, 512), main_chunk=2048,
                 chunk_bufs=3, wbufs=8, warmup_mms=12, pf=12, warmup_fd=128)
V3_OPTS_H = dict(warm=(128, 256, 512), main_chunk=2048,
                 chunk_bufs=3, wbufs=8, warmup_mms=18, pf=12, warmup_fd=128)
V3_OPTS_I = dict(warm=(128, 256, 512), main_chunk=2048,
                 chunk_bufs=3, wbufs=8, warmup_mms=8, pf=12, split_mt=True)
V3_OPTS_J = dict(warm=(128, 256, 512), main_chunk=2048,
                 chunk_bufs=3, wbufs=8, warmup_mms=5, pf=4, split_mt="fine")
V3_OPTS_K = dict(warm=(128, 256, 512), main_chunk=2048,
                 chunk_bufs=3, wbufs=8, warmup_mms=8, pf=12, split_mt=True,
                 alt_out_queue=True)
V3_OPTS_L = dict(warm=(128, 256, 512), main_chunk=2048,
                 chunk_bufs=4, wbufs=8, warmup_mms=7, pf=6, split_mt=True)
V3_OPTS_M = dict(warm=(128, 256, 512), main_chunk=2048,
                 chunk_bufs=3, wbufs=8, warmup_mms=8, pf=12, split_mt=True,
                 fast_tail=2)
V3_OPTS_N = dict(warm=(128, 256, 512), main_chunk=2048,
                 chunk_bufs=3, wbufs=8, warmup_mms=7, pf=4, split_mt=True)
V3_OPTS_P = dict(warm=(128, 256, 512), main_chunk=2048,
                 chunk_bufs=3, wbufs=8, warmup_mms=11, pf=12, split_mt="h",
                 warmup_noinit=True)
V3_OPTS_Q = dict(warm=(128, 256, 512), main_chunk=2048,
                 chunk_bufs=3, wbufs=8, warmup_mms=6, pf=12, split_mt=True)
V3_OPTS_R = dict(warm=(128, 256, 512), main_chunk=2048,
                 chunk_bufs=3, wbufs=8, warmup_mms=4, pf=12, split_mt=True)
V3_OPTS_S = dict(warm=(128, 256, 512), main_chunk=2048,
                 chunk_bufs=3, wbufs=8, warmup_mms=6, pf=12, split_mt="h")
V3_OPTS_T = dict(warm=(128, 256, 512), main_chunk=2048,
                 chunk_bufs=3, wbufs=8, warmup_mms=5, pf=12, split_mt="h")


def _active_opts():
    return {5: V3_OPTS_C, 6: V3_OPTS_D, 7: V3_OPTS_E, 8: V3_OPTS_F,
            9: V3_OPTS_G, 10: V3_OPTS_H, 11: V3_OPTS_I, 12: V3_OPTS_J,
            13: V3_OPTS_K, 14: V3_OPTS_L, 15: V3_OPTS_M, 16: V3_OPTS_N,
            17: V3_OPTS_P, 18: V3_OPTS_Q, 19: V3_OPTS_R, 20: V3_OPTS_S,
            21: V3_OPTS_T, 30: V4_OPTS_A, 31: V5_OPTS_A, 40: V6_OPTS_A,
            41: V6_OPTS_B, 50: V7_OPTS_A, 51: V7_OPTS_B, 52: V7_OPTS_C,
            53: V7_OPTS_D, 54: V7_OPTS_E, 55: V7_OPTS_F,
            4: V3_OPTS_B, 3: V3_OPTS}.get(VERSION, V3_OPTS_C)


def get_nc(reps=1, out_u8=True):
    if VERSION >= 50:
        opts = _active_opts()
        key = (VERSION, reps, tuple(sorted(opts.items())), out_u8)
        if key not in _nc_cache:
            _nc_cache[key] = _build_nc_v7(reps, out_u8=out_u8, **opts)
        return _nc_cache[key]
    if VERSION >= 40:
        opts = _active_opts()
        key = (VERSION, reps, tuple(sorted(opts.items())), out_u8)
        if key not in _nc_cache:
            _nc_cache[key] = _build_nc_v6(reps, out_u8=out_u8, **opts)
        return _nc_cache[key]
    if VERSION >= 30:
        opts = _active_opts()
        key = (VERSION, reps, tuple(sorted(opts.items())), out_u8)
        if key not in _nc_cache:
            _nc_cache[key] = _build_nc_v4(reps, out_u8=out_u8, **opts)
        return _nc_cache[key]
    if VERSION in (9, 10, 11, 12, 13, 14, 15, 16, 17, 18, 19, 20, 21):
        opts = _active_opts()
        key = (VERSION, reps, tuple(sorted(opts.items())), out_u8)
        if key not in _nc_cache:
            _nc_cache[key] = _build_nc_v3(reps, out_u8=out_u8, **opts)
        return _nc_cache[key]
    if VERSION == 8:
        key = (8, reps, tuple(sorted(V3_OPTS_F.items())), out_u8)
        if key not in _nc_cache:
            _nc_cache[key] = _build_nc_v3(reps, out_u8=out_u8, **V3_OPTS_F)
        return _nc_cache[key]
    if VERSION == 7:
        key = (7, reps, tuple(sorted(V3_OPTS_E.items())), out_u8)
        if key not in _nc_cache:
            _nc_cache[key] = _build_nc_v3(reps, out_u8=out_u8, **V3_OPTS_E)
        return _nc_cache[key]
    if VERSION == 6:
        key = (6, reps, tuple(sorted(V3_OPTS_D.items())), out_u8)
        if key not in _nc_cache:
            _nc_cache[key] = _build_nc_v3(reps, out_u8=out_u8, **V3_OPTS_D)
        return _nc_cache[key]
    if VERSION == 5:
        key = (5, reps, tuple(sorted(V3_OPTS_C.items())), out_u8)
        if key not in _nc_cache:
            _nc_cache[key] = _build_nc_v3(reps, out_u8=out_u8, **V3_OPTS_C)
        return _nc_cache[key]
    if VERSION == 4:
        key = (4, reps, tuple(sorted(V3_OPTS_B.items())), out_u8)
        if key not in _nc_cache:
            _nc_cache[key] = _build_nc_v3(reps, out_u8=out_u8, **V3_OPTS_B)
        return _nc_cache[key]
    if VERSION == 3:
        key = (3, reps, tuple(sorted(V3_OPTS.items())), out_u8)
        if key not in _nc_cache:
            _nc_cache[key] = _build_nc_v3(reps, out_u8=out_u8, **V3_OPTS)
        return _nc_cache[key]
    if VERSION == 2:
        key = (2, reps, tuple(sorted(V2_OPTS.items())))
        if key not in _nc_cache:
            _nc_cache[key] = _build_nc_v2(reps, **V2_OPTS)
        return _nc_cache[key]
    key = (reps, EVICT, W1_ACT, U_CHUNKS, WBUFS, KS_OUTER)
    if key not in _nc_cache:
        _nc_cache[key] = _build_nc(reps, evict=EVICT, w1_act=W1_ACT,
                                   u_chunks=U_CHUNKS, wbufs=WBUFS,
                                   ks_outer=KS_OUTER)
    return _nc_cache[key]


def _to_k_major(a_km, free):
    """[K, free] -> [P, KT, free] with k = ks*128 + p."""
    return np.ascontiguousarray(
        a_km.reshape(KT, P, free).transpose(1, 0, 2)
    )


def make_in_maps(u, M):
    u8 = np.asarray(u).astype(FP8_NP)
    m8 = np.asarray(M).astype(FP8_NP)
    mat3 = _to_k_major(m8, N)
    if VERSION >= 50:
        # v7 blob: [c0 | mt01 | mt23 | c1 | c2 | c3 | c4 | c5 | c6]
        starts = [0, 128, 384, 896, 2944, 4992, 7040]
        widths = [128, 256, 512, 2048, 2048, 2048, 1152]
        mt01 = mat3[:, 0:2, :].reshape(P, 2 * N)
        mt23 = mat3[:, 2:4, :].reshape(P, 2 * N)
        in_maps = []
        for i in range(N_CORES):
            uT_i = np.ascontiguousarray(u8[i * SHARD:(i + 1) * SHARD, :].T)
            uk = _to_k_major(uT_i, SHARD)
            ch = [uk[:, :, st:st + cw].reshape(P, KT * cw)
                  for st, cw in zip(starts, widths)]
            blob = np.concatenate(
                [ch[0], mt01, mt23, ch[1], ch[2], ch[3], ch[4], ch[5],
                 ch[6]], axis=1)
            assert blob.shape == (P, V7_TOTAL)
            in_maps.append({"blob": np.ascontiguousarray(blob)})
        return in_maps
    opts = _active_opts() if VERSION >= 3 else {}
    chunk_major = bool(opts.get("chunk_major")) or VERSION >= 40
    if chunk_major:
        starts, chunks = chunk_schedule(opts["warm"], opts["main_chunk"])
    in_maps = []
    for i in range(N_CORES):
        uT_i = np.ascontiguousarray(u8[i * SHARD:(i + 1) * SHARD, :].T)
        uk = _to_k_major(uT_i, SHARD)  # [P, KT, SHARD]
        if chunk_major:
            uk = np.concatenate(
                [uk[:, :, st:st + cw].reshape(P, KT * cw)
                 for st, cw in zip(starts, chunks)], axis=1)
        in_maps.append({"uT": uk, "mat": mat3})
    return in_maps


def ungroup_y(yc, group):
    """[NB/group, P, group*N] grouped layout -> [SHARD, N]."""
    ng = NB // group
    return np.ascontiguousarray(
        yc.reshape(ng, P, group, N).transpose(0, 2, 1, 3).reshape(SHARD, N))


def kernel(u, crc_gen, info_pos, ind_gather, perm_out):
    from concourse.bass_utils import run_bass_kernel_spmd

    M = build_M(crc_gen, info_pos, ind_gather, perm_out)
    in_maps = make_in_maps(u, M)
    nc = get_nc()
    res = run_bass_kernel_spmd(nc, in_maps, core_ids=list(range(N_CORES)))
    ys = [np.asarray(r["y"]) for r in res.results]
    if ys[0].dtype == np.uint8 and any((yc == 255).any() for yc in ys):
        # saturation certificate failed (a sum may have clipped at 255):
        # rerun with exact i16 output
        nc16 = get_nc(out_u8=False)
        res = run_bass_kernel_spmd(nc16, in_maps,
                                   core_ids=list(range(N_CORES)))
        ys = [np.asarray(r["y"]) for r in res.results]
    group = _active_opts().get("group") if VERSION >= 40 else None
    if group:
        ys = [ungroup_y(yc, group) for yc in ys]
    out = np.concatenate([(yc & 1).astype(np.float32) for yc in ys], axis=0)
    return out

